# revision 1
# baseline (speedup 1.0000x reference)
"""8-core Trainium2 Bass kernel for nn_MixModel (GCN mix model) — v2.

Sharding: nodes dealt round-robin by in-degree rank to 8 cores; each core owns
NLOC = ceil((ceil(N/8)+1)/128)*128 local rows (>=1 zero pad row reused as the
ELL gather-pad target).

Algebra used:
 - GCN messages factorize: msg = (h*dis)[src], output scaled by dis[dst]; the
   self-loop is one extra ELL slot. Aggregation = unweighted padded-ELL
   gather+sum of pre-scaled table rows.
 - segsum and the layer matmul commute: sum((z@W*dis)[src]) =
   sum((z*dis)[src]) @ W — so cores AllGather the *scaled activations* and the
   per-layer matmul runs on the 98 aggregated dst tiles inside the gather
   pipeline (hidden under Q7 descriptor generation), not on 784 table tiles.
 - good/bad paths share edge sets -> gather concatenated 256-wide tables.
 - the permuted-input path's first-layer table is a cheap local permutation
   gather of the xW1' table (12.5k rows), not a per-edge pass.

Stages (per core):
  S0   xW1' shard = (x_sh @ W1) * dis_sh
  AG0  AllGather -> XW [NG,128]
  S2   T1 shard = [xW1'_loc | gather(XW, gperm)*ratio] ; AG1 -> T1 [NG,256]
  G1   ELL gather T1 -> zd = relu(dis^2 * sum)  (= z1*dis)      -> AG2 ZD
  G2   ELL gather ZD -> S ; e1{,b} = relu(dis * (S_h @ W2)) ;
       ship [e1*dish|e1b*dish] -> AG3a E1H ; [e1*dis] -> AG3b E1D ; e1 local
  G3   ELL gather E1H (hop order) -> embed2{,b} = dish * (S_h @ W3) -> E2h
  S12  MLP: embed3 = relu(e1@M1)@M2 ; tvec = embed3@Wd0
  S11  realign E2h to pi order ; scores = sigmoid(rowsum(tvec * e2{,b}))
  G4   ELL gather E1D -> cls = (dis*sum)@Wc -> OUT[:, :10]
"""

import numpy as np

import concourse.bacc as bacc
import concourse.bass as bass
import concourse.mybir as mybir
import concourse.tile as tile
from concourse import bass_utils
from concourse.masks import make_identity

P = 128
F32 = mybir.dt.float32
I32 = mybir.dt.int32
AF = mybir.ActivationFunctionType
ALU = mybir.AluOpType
TDT = mybir.dt.bfloat16  # transport/table dtype


# ----------------------------------------------------------------- host prep


def _ell_build(src_g, dst_core, dst_loc, self_g, n_cores, nloc, padrow):
    """Shared-K ELL: returns (K per tile, per-core int32 [P, sum(K)] arrays,
    p-major-global: element [p, koff[t]+k] = slot k of local row t*128+p)."""
    nt = nloc // P
    counts = np.zeros((n_cores, nloc), np.int64)
    np.add.at(counts, (dst_core, dst_loc), 1)
    n_self = 0 if self_g is None else 1
    cmax = counts.reshape(n_cores, nt, P).max(axis=(0, 2))
    K = (cmax + n_self).astype(np.int64)
    order = np.lexsort((dst_loc, dst_core))
    sc, sl, sg = dst_core[order], dst_loc[order], src_g[order]
    key = sc.astype(np.int64) * nloc + sl
    is_start = np.r_[True, key[1:] != key[:-1]] if len(key) else np.array([], bool)
    run_starts = np.flatnonzero(is_start)
    run_len = np.diff(np.r_[run_starts, len(key)])
    pos_in_run = np.arange(len(key)) - np.repeat(run_starts, run_len)
    koff = np.r_[0, np.cumsum(K)]
    sk = int(koff[-1])
    idx_arrs = []
    for c in range(n_cores):
        arr = np.full((P, sk), padrow[c], np.int64)
        m = sc == c
        loc, pos, gidx = sl[m], pos_in_run[m], sg[m]
        t = loc // P
        p = loc % P
        arr[p, koff[t] + pos + n_self] = gidx
        if n_self:
            allt = np.arange(nloc) // P
            allp = np.arange(nloc) % P
            arr[allp, koff[allt]] = self_g[c]
        idx_arrs.append(arr.astype(np.int32))
    return K.tolist(), idx_arrs


def _plane(vals_loc, nt):
    """[nloc] local-row vector -> [P, nt] plane (local row t*128+p -> [p, t])."""
    return np.ascontiguousarray(vals_loc.reshape(nt, P).T)


def prep(inputs, n_cores=8):
    x = np.asarray(inputs["x"], np.float32)
    ei = np.asarray(inputs["edge_index"], np.int64)
    eih = np.asarray(inputs["edge_index_hop"], np.int64)
    perm = np.asarray(inputs["perm"], np.int64)
    W1 = np.asarray(inputs["W1"], np.float32)
    W2 = np.asarray(inputs["W2"], np.float32)
    W3 = np.asarray(inputs["W3"], np.float32)
    M1 = np.asarray(inputs["M1"], np.float32)
    M2 = np.asarray(inputs["M2"], np.float32)
    Wc = np.asarray(inputs["Wc"], np.float32)
    Wd0 = np.asarray(inputs["Wd"], np.float32)[0]
    for bname in ("b1", "b2", "b3", "mb1", "mb2", "bc"):
        assert np.abs(np.asarray(inputs[bname])).max() == 0.0, (
            f"nonzero bias {bname} not supported by this kernel build"
        )

    N, n_feat = x.shape
    D = W1.shape[1]
    ncls = Wc.shape[1]
    max_real = -(-N // n_cores)
    nloc = -(-(max_real + 1) // P) * P
    nt = nloc // P
    ng = n_cores * nloc

    deg = np.bincount(ei[1], minlength=N).astype(np.float32) + 1.0
    degh = np.bincount(eih[1], minlength=N).astype(np.float32) + 1.0
    dis = 1.0 / np.sqrt(deg)
    dish = 1.0 / np.sqrt(degh)

    order = np.argsort(-deg, kind="stable")
    core_of = np.empty(N, np.int64)
    loc_of = np.empty(N, np.int64)
    core_of[order] = np.arange(N) % n_cores
    loc_of[order] = np.arange(N) // n_cores
    gl = core_of * nloc + loc_of
    padrow = [c * nloc + nloc - 1 for c in range(n_cores)]

    nat = np.full((n_cores, nloc), -1, np.int64)
    nat[core_of, loc_of] = np.arange(N)

    # hop order: per-core resort by hop degree desc (pads last)
    hkey = np.where(nat >= 0, -degh[np.maximum(nat, 0)], 1.0)
    hord = np.argsort(hkey, axis=1, kind="stable")
    hpos = np.argsort(hord, axis=1)

    selfg_pi = np.where(
        nat >= 0,
        np.arange(n_cores)[:, None] * nloc + np.arange(nloc)[None, :],
        np.array(padrow)[:, None],
    )
    K1, idx1 = _ell_build(
        gl[ei[0]], core_of[ei[1]], loc_of[ei[1]], selfg_pi, n_cores, nloc, padrow
    )
    selfg_h = np.take_along_axis(selfg_pi, hord, axis=1)
    K3, idx3 = _ell_build(
        gl[eih[0]],
        core_of[eih[1]],
        hpos[core_of[eih[1]], loc_of[eih[1]]],
        selfg_h,
        n_cores,
        nloc,
        padrow,
    )

    in_maps = []
    for c in range(n_cores):
        natc = nat[c]
        real = natc >= 0
        xs = np.zeros((nloc, n_feat), np.float32)
        xs[real] = x[natc[real]]
        dis_c = np.ones(nloc, np.float32)
        dis_c[real] = dis[natc[real]]
        dish_pi = np.ones(nloc, np.float32)
        dish_pi[real] = dish[natc[real]]
        dishh = np.ones(nloc, np.float32)
        hnat = natc[hord[c]]
        hreal = hnat >= 0
        dishh[hreal] = dish[hnat[hreal]]
        gperm = np.full(nloc, padrow[c], np.int64)
        ratio = np.ones(nloc, np.float32)
        pv = perm[natc[real]]
        gperm[real] = gl[pv]
        ratio[real] = dis[natc[real]] / dis[pv]
        in_maps.append(
            {
                "xT": np.ascontiguousarray(xs.T),
                "dis_p": _plane(dis_c, nt),
                "dis2_p": _plane(dis_c * dis_c, nt),
                "dishp_p": _plane(dish_pi, nt),
                "dishh_p": _plane(dishh, nt),
                "ratio_p": _plane(ratio, nt),
                "gperm_p": _plane(gperm.astype(np.int32), nt),
                "idxR_p": _plane(hpos[c].astype(np.int32), nt),
                "idx1": idx1[c],
                "idx3": idx3[c],
                "W1": W1,
                "W2": W2,
                "W3": W3,
                "M1": M1,
                "M2": M2,
                "Wd0": Wd0,
                "Wc": np.ascontiguousarray(Wc),
            }
        )

    meta = dict(
        n_cores=n_cores,
        nloc=nloc,
        nt=nt,
        ng=ng,
        n_feat=n_feat,
        D=D,
        ncls=ncls,
        K1=K1,
        K3=K3,
        core_of=core_of,
        loc_of=loc_of,
    )
    return in_maps, meta


# ------------------------------------------------------------- device build


def build(meta):
    n_cores = meta["n_cores"]
    nloc, nt, ng = meta["nloc"], meta["nt"], meta["ng"]
    n_feat, D, ncls = meta["n_feat"], meta["D"], meta["ncls"]
    K1, K3 = meta["K1"], meta["K3"]
    DD = 2 * D
    nfc = n_feat // P
    sk1, sk3 = sum(K1), sum(K3)
    groups = [list(range(n_cores))]

    nc = bacc.Bacc("TRN2", debug=False, num_devices=n_cores)
    shared = "Shared" if n_cores > 4 else "Local"

    xT = nc.dram_tensor("xT", [n_feat, nloc], F32, kind="ExternalInput")
    dis_p = nc.dram_tensor("dis_p", [P, nt], F32, kind="ExternalInput")
    dis2_p = nc.dram_tensor("dis2_p", [P, nt], F32, kind="ExternalInput")
    dishp_p = nc.dram_tensor("dishp_p", [P, nt], F32, kind="ExternalInput")
    dishh_p = nc.dram_tensor("dishh_p", [P, nt], F32, kind="ExternalInput")
    ratio_p = nc.dram_tensor("ratio_p", [P, nt], F32, kind="ExternalInput")
    gperm_p = nc.dram_tensor("gperm_p", [P, nt], I32, kind="ExternalInput")
    idxR_p = nc.dram_tensor("idxR_p", [P, nt], I32, kind="ExternalInput")
    idx1 = nc.dram_tensor("idx1", [P, sk1], I32, kind="ExternalInput")
    idx3 = nc.dram_tensor("idx3", [P, sk3], I32, kind="ExternalInput")
    W1 = nc.dram_tensor("W1", [n_feat, D], F32, kind="ExternalInput")
    W2 = nc.dram_tensor("W2", [D, D], F32, kind="ExternalInput")
    W3 = nc.dram_tensor("W3", [D, D], F32, kind="ExternalInput")
    M1 = nc.dram_tensor("M1", [D, D], F32, kind="ExternalInput")
    M2 = nc.dram_tensor("M2", [D, D], F32, kind="ExternalInput")
    Wd0 = nc.dram_tensor("Wd0", [D, D], F32, kind="ExternalInput")
    Wc = nc.dram_tensor("Wc", [D, ncls], F32, kind="ExternalInput")
    out = nc.dram_tensor("out", [nloc, ncls + 2], F32, kind="ExternalOutput")

    xw_s = nc.dram_tensor("xw_s", [nloc, D], TDT, kind="Internal")
    XW = nc.dram_tensor("XW", [ng, D], TDT, kind="Internal", addr_space=shared)
    t1_s = nc.dram_tensor("t1_s", [nloc, DD], TDT, kind="Internal")
    T1 = nc.dram_tensor("T1", [ng, DD], TDT, kind="Internal", addr_space=shared)
    zd_s = nc.dram_tensor("zd_s", [nloc, DD], TDT, kind="Internal")
    ZD = nc.dram_tensor("ZD", [ng, DD], TDT, kind="Internal", addr_space=shared)
    e1_s = nc.dram_tensor("e1_s", [nloc, D], F32, kind="Internal")
    e1h_s = nc.dram_tensor("e1h_s", [nloc, DD], TDT, kind="Internal")
    e1d_s = nc.dram_tensor("e1d_s", [nloc, D], TDT, kind="Internal")
    E1H = nc.dram_tensor("E1H", [ng, DD], TDT, kind="Internal", addr_space=shared)
    E1D = nc.dram_tensor("E1D", [ng, D], TDT, kind="Internal", addr_space=shared)
    E2h = nc.dram_tensor("E2h", [nloc, DD], F32, kind="Internal")
    TV = nc.dram_tensor("TV", [nloc, D], F32, kind="Internal")

    with tile.TileContext(nc) as tc:
        with (
            tc.tile_pool(name="const", bufs=1) as constp,
            tc.tile_pool(name="gath", bufs=3) as gathp,
            tc.tile_pool(name="work", bufs=3) as workp,
            tc.tile_pool(name="outp", bufs=3) as outp,
            tc.tile_pool(name="psum", bufs=2, space="PSUM") as psp,
        ):
            ident = constp.tile([P, P], F32)
            make_identity(nc, ident[:])

            # resident planes + indices
            def res(t_dram, w, dt=F32, name=None):
                tl = constp.tile([P, w], dt, name=name)
                nc.sync.dma_start(tl[:], t_dram.ap())
                return tl

            disq = res(dis_p, nt, name="disq")
            dis2q = res(dis2_p, nt, name="dis2q")
            dishpq = res(dishp_p, nt, name="dishpq")
            dishhq = res(dishh_p, nt, name="dishhq")
            ratioq = res(ratio_p, nt, name="ratioq")
            gpermq = res(gperm_p, nt, I32, name="gpermq")
            idxRq = res(idxR_p, nt, I32, name="idxRq")
            idx1q = res(idx1, sk1, I32, name="idx1q")
            idx3q = res(idx3, sk3, I32, name="idx3q")

            w1t = [
                constp.tile([P, D], F32, name=f"w1t_{i}") for i in range(nfc)
            ]
            for i in range(nfc):
                nc.sync.dma_start(w1t[i][:], W1.ap()[i * P : (i + 1) * P])
            w2t = res(W2, D, name="w2t")
            w3t = res(W3, D, name="w3t")
            m1t = res(M1, D, name="m1t")
            m2t = res(M2, D, name="m2t")
            wdt = res(Wd0, D, name="wdt")
            wct = res(Wc, ncls, name="wct")

            def rows(t):
                return slice(t * P, (t + 1) * P)

            def col(plane, t):
                return plane[:, t : t + 1]

            # ---- S0: xW1' shard
            for t in range(nt):
                ps = psp.tile([P, D], F32, tag="mm")
                for i in range(nfc):
                    xt = workp.tile([P, P], F32, tag="xt")
                    nc.sync.dma_start(xt[:], xT.ap()[i * P : (i + 1) * P, rows(t)])
                    nc.tensor.matmul(
                        out=ps[:],
                        lhsT=xt[:],
                        rhs=w1t[i][:],
                        start=(i == 0),
                        stop=(i == nfc - 1),
                    )
                o = outp.tile([P, D], TDT, tag="s0")
                nc.vector.tensor_scalar_mul(o[:], ps[:], col(disq, t))
                nc.sync.dma_start(xw_s.ap()[rows(t)], o[:])

            nc.gpsimd.collective_compute(
                "AllGather",
                ALU.bypass,
                replica_groups=groups,
                ins=[xw_s[:].opt()],
                outs=[XW[:].opt()],
            )

            # ---- S2: T1 shard
            for t in range(nt):
                g = gathp.tile([P, D], TDT, tag="g2")
                nc.gpsimd.indirect_dma_start(
                    out=g[:],
                    out_offset=None,
                    in_=XW.ap(),
                    in_offset=bass.IndirectOffsetOnAxis(ap=col(gpermq, t), axis=0),
                )
                o = outp.tile([P, D], TDT, tag="s2")
                nc.vector.tensor_scalar_mul(o[:], g[:], col(ratioq, t))
                nc.sync.dma_start(t1_s.ap()[rows(t), D:DD], o[:])
                l = workp.tile([P, D], TDT, tag="s2l")
                nc.sync.dma_start(l[:], xw_s.ap()[rows(t)])
                nc.sync.dma_start(t1_s.ap()[rows(t), 0:D], l[:])

            nc.gpsimd.collective_compute(
                "AllGather",
                ALU.bypass,
                replica_groups=groups,
                ins=[t1_s[:].opt()],
                outs=[T1[:].opt()],
            )

            # ---- generic ELL gather: returns aggregated [P, width] tile
            def ell_tile(table, width, K, koff, idxq):
                g = gathp.tile([P, K * width], TDT, tag="ge")
                for k in range(K):
                    nc.gpsimd.indirect_dma_start(
                        out=g[:, k * width : (k + 1) * width],
                        out_offset=None,
                        in_=table.ap(),
                        in_offset=bass.IndirectOffsetOnAxis(
                            ap=idxq[:, koff + k : koff + k + 1], axis=0
                        ),
                    )
                s = workp.tile([P, width], F32, tag="se")
                if K == 1:
                    nc.vector.tensor_copy(s[:], g[:])
                else:
                    g3 = g[:].rearrange("p (k d) -> p d k", k=K)
                    nc.vector.tensor_reduce(
                        out=s[:], in_=g3, axis=mybir.AxisListType.X, op=ALU.add
                    )
                return s

            # ---- G1: zd = relu(dis2 * sum) -> zd_s
            koff = 0
            for t in range(nt):
                s = ell_tile(T1, DD, K1[t], koff, idx1q)
                koff += K1[t]
                o = outp.tile([P, DD], TDT, tag="ze")
                nc.vector.tensor_scalar(
                    o[:], s[:], col(dis2q, t), 0.0, ALU.mult, ALU.max
                )
                nc.sync.dma_start(zd_s.ap()[rows(t)], o[:])

            nc.gpsimd.collective_compute(
                "AllGather",
                ALU.bypass,
                replica_groups=groups,
                ins=[zd_s[:].opt()],
                outs=[ZD[:].opt()],
            )

            # ---- G2: S @ W2, three shipped variants
            koff = 0
            for t in range(nt):
                s = ell_tile(ZD, DD, K1[t], koff, idx1q)
                koff += K1[t]
                e1h = outp.tile([P, DD], TDT, tag="e1h")
                e1d = outp.tile([P, D], TDT, tag="e1d")
                e1p = outp.tile([P, D], F32, tag="e1p")
                for h in range(2):
                    tp = psp.tile([P, P], F32, tag="t", bufs=3)
                    nc.tensor.transpose(
                        out=tp[:], in_=s[:, h * D : (h + 1) * D], identity=ident[:]
                    )
                    tps = workp.tile([P, P], F32, tag="tps")
                    nc.vector.tensor_copy(tps[:], tp[:])
                    mm = psp.tile([P, D], F32, tag="m", bufs=3)
                    nc.tensor.matmul(
                        out=mm[:], lhsT=tps[:], rhs=w2t[:], start=True, stop=True
                    )
                    # e1 = relu(dis * mm)
                    eh = workp.tile([P, D], F32, tag="eh")
                    nc.vector.tensor_scalar(
                        eh[:], mm[:], col(disq, t), 0.0, ALU.mult, ALU.max
                    )
                    nc.vector.tensor_scalar_mul(
                        e1h[:, h * D : (h + 1) * D], eh[:], col(dishpq, t)
                    )
                    if h == 0:
                        nc.vector.tensor_copy(e1p[:], eh[:])
                        nc.vector.tensor_scalar_mul(e1d[:], eh[:], col(disq, t))
                nc.sync.dma_start(e1_s.ap()[rows(t)], e1p[:])
                nc.sync.dma_start(e1h_s.ap()[rows(t)], e1h[:])
                nc.sync.dma_start(e1d_s.ap()[rows(t)], e1d[:])

            nc.gpsimd.collective_compute(
                "AllGather",
                ALU.bypass,
                replica_groups=groups,
                ins=[e1h_s[:].opt()],
                outs=[E1H[:].opt()],
            )
            nc.gpsimd.collective_compute(
                "AllGather",
                ALU.bypass,
                replica_groups=groups,
                ins=[e1d_s[:].opt()],
                outs=[E1D[:].opt()],
            )

            # ---- S12: MLP + tvec (local, overlaps with AG3/G3)
            for t in range(nt):
                et = workp.tile([P, D], F32, tag="ml_in")
                nc.sync.dma_start(et[:], e1_s.ap()[rows(t)])
                tp = psp.tile([P, P], F32, tag="t", bufs=3)
                nc.tensor.transpose(out=tp[:], in_=et[:], identity=ident[:])
                tps = workp.tile([P, P], F32, tag="tps")
                nc.vector.tensor_copy(tps[:], tp[:])
                mm = psp.tile([P, D], F32, tag="m", bufs=3)
                nc.tensor.matmul(out=mm[:], lhsT=tps[:], rhs=m1t[:], start=True, stop=True)
                u = workp.tile([P, D], F32, tag="ml_u")
                nc.scalar.activation(u[:], mm[:], AF.Relu)
                tp2 = psp.tile([P, P], F32, tag="t", bufs=3)
                nc.tensor.transpose(out=tp2[:], in_=u[:], identity=ident[:])
                tps2 = workp.tile([P, P], F32, tag="tps")
                nc.vector.tensor_copy(tps2[:], tp2[:])
                mm2 = psp.tile([P, D], F32, tag="m", bufs=3)
                nc.tensor.matmul(
                    out=mm2[:], lhsT=tps2[:], rhs=m2t[:], start=True, stop=True
                )
                e3 = workp.tile([P, D], F32, tag="ml_e3")
                nc.vector.tensor_copy(e3[:], mm2[:])
                tp3 = psp.tile([P, P], F32, tag="t", bufs=3)
                nc.tensor.transpose(out=tp3[:], in_=e3[:], identity=ident[:])
                tps3 = workp.tile([P, P], F32, tag="tps")
                nc.vector.tensor_copy(tps3[:], tp3[:])
                mm3 = psp.tile([P, D], F32, tag="m", bufs=3)
                nc.tensor.matmul(
                    out=mm3[:], lhsT=tps3[:], rhs=wdt[:], start=True, stop=True
                )
                tv = outp.tile([P, D], F32, tag="ml_tv")
                nc.vector.tensor_copy(tv[:], mm3[:])
                nc.sync.dma_start(TV.ap()[rows(t)], tv[:])

            # ---- G3: embed2{,b} = dishh * (S_h @ W3) -> E2h (hop order)
            koff = 0
            for t in range(nt):
                s = ell_tile(E1H, DD, K3[t], koff, idx3q)
                koff += K3[t]
                e2 = outp.tile([P, DD], F32, tag="e2")
                for h in range(2):
                    tp = psp.tile([P, P], F32, tag="t", bufs=3)
                    nc.tensor.transpose(
                        out=tp[:], in_=s[:, h * D : (h + 1) * D], identity=ident[:]
                    )
                    tps = workp.tile([P, P], F32, tag="tps")
                    nc.vector.tensor_copy(tps[:], tp[:])
                    mm = psp.tile([P, D], F32, tag="m", bufs=3)
                    nc.tensor.matmul(
                        out=mm[:], lhsT=tps[:], rhs=w3t[:], start=True, stop=True
                    )
                    nc.vector.tensor_scalar_mul(
                        e2[:, h * D : (h + 1) * D], mm[:], col(dishhq, t)
                    )
                nc.sync.dma_start(E2h.ap()[rows(t)], e2[:])

            # ---- S11 + S13: realign + scores
            for t in range(nt):
                e2 = gathp.tile([P, DD], F32, tag="gr")
                nc.gpsimd.indirect_dma_start(
                    out=e2[:],
                    out_offset=None,
                    in_=E2h.ap(),
                    in_offset=bass.IndirectOffsetOnAxis(ap=col(idxRq, t), axis=0),
                )
                tv = workp.tile([P, D], F32, tag="sc_tv")
                nc.sync.dma_start(tv[:], TV.ap()[rows(t)])
                pr = workp.tile([P, DD], F32, tag="sc_pr")
                nc.vector.tensor_mul(pr[:, 0:D], tv[:], e2[:, 0:D])
                nc.vector.tensor_mul(pr[:, D:DD], tv[:], e2[:, D:DD])
                rs = workp.tile([P, 2], F32, tag="sc_rs")
                nc.vector.tensor_reduce(
                    out=rs[:],
                    in_=pr[:].rearrange("p (h d) -> p h d", h=2),
                    axis=mybir.AxisListType.X,
                    op=ALU.add,
                )
                sg = outp.tile([P, 2], F32, tag="sc_sg")
                nc.scalar.activation(sg[:], rs[:], AF.Sigmoid)
                nc.sync.dma_start(out.ap()[rows(t), ncls : ncls + 2], sg[:])

            # ---- G4: cls = (dis * sum) @ Wc -> out[:, :ncls]
            koff = 0
            for t in range(nt):
                s = ell_tile(E1D, D, K1[t], koff, idx1q)
                koff += K1[t]
                sc_ = workp.tile([P, D], F32, tag="c_s")
                nc.vector.tensor_scalar_mul(sc_[:], s[:], col(disq, t))
                tp = psp.tile([P, P], F32, tag="t", bufs=3)
                nc.tensor.transpose(out=tp[:], in_=sc_[:], identity=ident[:])
                tps = workp.tile([P, P], F32, tag="tps")
                nc.vector.tensor_copy(tps[:], tp[:])
                mm = psp.tile([P, ncls], F32, tag="m", bufs=3)
                nc.tensor.matmul(out=mm[:], lhsT=tps[:], rhs=wct[:], start=True, stop=True)
                o = outp.tile([P, ncls], F32, tag="c_o")
                nc.vector.tensor_copy(o[:], mm[:])
                nc.sync.dma_start(out.ap()[rows(t), 0:ncls], o[:])

    nc.compile()
    return nc


def assemble(results, meta):
    n_cores = meta["n_cores"]
    N = len(meta["core_of"])
    ncls = meta["ncls"]
    out = np.empty((N, ncls + 2), np.float32)
    for c in range(n_cores):
        oc = results[c]["out"]
        m = meta["core_of"] == c
        out[m] = oc[meta["loc_of"][m]]
    return out


# ------------------------------------------------------------------ entry


_CACHE = {}


def kernel(**inputs):
    """Full-input entry point: shards across 8 NeuronCores internally.

    Expects the nn_MixModel input dict (x, edge_index, edge_index_hop, y,
    perm, W1..Wd); returns the full [N, n_cls+2] float32 output.
    """
    n_cores = 8
    in_maps, meta = prep(inputs, n_cores)
    key = (meta["nloc"], tuple(meta["K1"]), tuple(meta["K3"]))
    nc = _CACHE.get(key)
    if nc is None:
        nc = build(meta)
        _CACHE[key] = nc
    res = bass_utils.run_bass_kernel_spmd(
        nc, in_maps, core_ids=list(range(n_cores)), trace=False
    )
    return assemble(res.results, meta)



# revision 12
# speedup vs baseline: 1.0223x; 1.0223x over previous
"""8-core Trainium2 Bass kernel for nn_MixModel (GCN mix model) — v2.

Sharding: nodes dealt round-robin by in-degree rank to 8 cores; each core owns
NLOC = ceil((ceil(N/8)+1)/128)*128 local rows (>=1 zero pad row reused as the
ELL gather-pad target).

Algebra used:
 - GCN messages factorize: msg = (h*dis)[src], output scaled by dis[dst]; the
   self-loop is one extra ELL slot. Aggregation = unweighted padded-ELL
   gather+sum of pre-scaled table rows.
 - segsum and the layer matmul commute: sum((z@W*dis)[src]) =
   sum((z*dis)[src]) @ W — so cores AllGather the *scaled activations* and the
   per-layer matmul runs on the 98 aggregated dst tiles inside the gather
   pipeline (hidden under Q7 descriptor generation), not on 784 table tiles.
 - good/bad paths share edge sets -> gather concatenated 256-wide tables.
 - the permuted-input path's first-layer table is a cheap local permutation
   gather of the xW1' table (12.5k rows), not a per-edge pass.

Stages (per core):
  S0   xW1' shard = (x_sh @ W1) * dis_sh
  AG0  AllGather -> XW [NG,128]
  S2   T1 shard = [xW1'_loc | gather(XW, gperm)*ratio] ; AG1 -> T1 [NG,256]
  G1   ELL gather T1 -> zd = relu(dis^2 * sum)  (= z1*dis)      -> AG2 ZD
  G2   ELL gather ZD -> S ; e1{,b} = relu(dis * (S_h @ W2)) ;
       ship [e1*dish|e1b*dish] -> AG3a E1H ; [e1*dis] -> AG3b E1D ; e1 local
  G3   ELL gather E1H (hop order) -> embed2{,b} = dish * (S_h @ W3) -> E2h
  S12  MLP: embed3 = relu(e1@M1)@M2 ; tvec = embed3@Wd0
  S11  realign E2h to pi order ; scores = sigmoid(rowsum(tvec * e2{,b}))
  G4   ELL gather E1D -> cls = (dis*sum)@Wc -> OUT[:, :10]
"""

import numpy as np

import concourse.bacc as bacc
import concourse.bass as bass
import concourse.mybir as mybir
import concourse.tile as tile
from concourse import bass_utils
from concourse.masks import make_identity

P = 128
F32 = mybir.dt.float32
I32 = mybir.dt.int32
AF = mybir.ActivationFunctionType
ALU = mybir.AluOpType
TDT = mybir.dt.bfloat16  # transport/table dtype


# ----------------------------------------------------------------- host prep


def _ell_build(src_g, dst_core, dst_loc, self_g, n_cores, nloc, padrow):
    """Shared-K ELL: returns (K per tile, per-core int32 [P, sum(K)] arrays,
    p-major-global: element [p, koff[t]+k] = slot k of local row t*128+p)."""
    nt = nloc // P
    counts = np.zeros((n_cores, nloc), np.int64)
    np.add.at(counts, (dst_core, dst_loc), 1)
    n_self = 0 if self_g is None else 1
    cmax = counts.reshape(n_cores, nt, P).max(axis=(0, 2))
    K = (cmax + n_self).astype(np.int64)
    order = np.lexsort((dst_loc, dst_core))
    sc, sl, sg = dst_core[order], dst_loc[order], src_g[order]
    key = sc.astype(np.int64) * nloc + sl
    is_start = np.r_[True, key[1:] != key[:-1]] if len(key) else np.array([], bool)
    run_starts = np.flatnonzero(is_start)
    run_len = np.diff(np.r_[run_starts, len(key)])
    pos_in_run = np.arange(len(key)) - np.repeat(run_starts, run_len)
    koff = np.r_[0, np.cumsum(K)]
    sk = int(koff[-1])
    idx_arrs = []
    for c in range(n_cores):
        arr = np.full((P, sk), padrow[c], np.int64)
        m = sc == c
        loc, pos, gidx = sl[m], pos_in_run[m], sg[m]
        t = loc // P
        p = loc % P
        arr[p, koff[t] + pos + n_self] = gidx
        if n_self:
            allt = np.arange(nloc) // P
            allp = np.arange(nloc) % P
            arr[allp, koff[allt]] = self_g[c]
        idx_arrs.append(arr.astype(np.int32))
    return K.tolist(), idx_arrs


def _plane(vals_loc, nt):
    """[nloc] local-row vector -> [P, nt] plane (local row t*128+p -> [p, t])."""
    return np.ascontiguousarray(vals_loc.reshape(nt, P).T)


def prep(inputs, n_cores=8):
    x = np.asarray(inputs["x"], np.float32)
    ei = np.asarray(inputs["edge_index"], np.int64)
    eih = np.asarray(inputs["edge_index_hop"], np.int64)
    perm = np.asarray(inputs["perm"], np.int64)
    W1 = np.asarray(inputs["W1"], np.float32)
    W2 = np.asarray(inputs["W2"], np.float32)
    W3 = np.asarray(inputs["W3"], np.float32)
    M1 = np.asarray(inputs["M1"], np.float32)
    M2 = np.asarray(inputs["M2"], np.float32)
    Wc = np.asarray(inputs["Wc"], np.float32)
    Wd0 = np.asarray(inputs["Wd"], np.float32)[0]
    for bname in ("b1", "b2", "b3", "mb1", "mb2", "bc"):
        assert np.abs(np.asarray(inputs[bname])).max() == 0.0, (
            f"nonzero bias {bname} not supported by this kernel build"
        )

    N, n_feat = x.shape
    D = W1.shape[1]
    ncls = Wc.shape[1]
    max_real = -(-N // n_cores)
    nloc = -(-(max_real + 1) // P) * P
    nt = nloc // P
    ng = n_cores * nloc

    deg = np.bincount(ei[1], minlength=N).astype(np.float32) + 1.0
    degh = np.bincount(eih[1], minlength=N).astype(np.float32) + 1.0
    dis = 1.0 / np.sqrt(deg)
    dish = 1.0 / np.sqrt(degh)

    order = np.argsort(-deg, kind="stable")
    core_of = np.empty(N, np.int64)
    loc_of = np.empty(N, np.int64)
    core_of[order] = np.arange(N) % n_cores
    loc_of[order] = np.arange(N) // n_cores
    gl = core_of * nloc + loc_of
    padrow = [c * nloc + nloc - 1 for c in range(n_cores)]

    nat = np.full((n_cores, nloc), -1, np.int64)
    nat[core_of, loc_of] = np.arange(N)

    # hop order: per-core resort by hop degree desc (pads last)
    hkey = np.where(nat >= 0, -degh[np.maximum(nat, 0)], 1.0)
    hord = np.argsort(hkey, axis=1, kind="stable")
    hpos = np.argsort(hord, axis=1)

    selfg_pi = np.where(
        nat >= 0,
        np.arange(n_cores)[:, None] * nloc + np.arange(nloc)[None, :],
        np.array(padrow)[:, None],
    )
    # graph-g passes add the self term via a direct DMA of the local shard
    # (saves one ELL slot per tile on 3 passes); hop pass keeps self in ELL
    # (hop order makes the local slice non-contiguous).
    K1, idx1 = _ell_build(
        gl[ei[0]], core_of[ei[1]], loc_of[ei[1]], None, n_cores, nloc, padrow
    )
    selfg_h = np.take_along_axis(selfg_pi, hord, axis=1)
    K3, idx3 = _ell_build(
        gl[eih[0]],
        core_of[eih[1]],
        hpos[core_of[eih[1]], loc_of[eih[1]]],
        selfg_h,
        n_cores,
        nloc,
        padrow,
    )

    in_maps = []
    for c in range(n_cores):
        natc = nat[c]
        real = natc >= 0
        xs = np.zeros((nloc, n_feat), np.float32)
        xs[real] = x[natc[real]]
        dis_c = np.ones(nloc, np.float32)
        dis_c[real] = dis[natc[real]]
        dish_pi = np.ones(nloc, np.float32)
        dish_pi[real] = dish[natc[real]]
        dishh = np.ones(nloc, np.float32)
        hnat = natc[hord[c]]
        hreal = hnat >= 0
        dishh[hreal] = dish[hnat[hreal]]
        gperm = np.full(nloc, padrow[c], np.int64)
        ratio = np.ones(nloc, np.float32)
        pv = perm[natc[real]]
        gperm[real] = gl[pv]
        ratio[real] = dis[natc[real]] / dis[pv]
        in_maps.append(
            {
                "xT": np.ascontiguousarray(xs.T),
                "dis_p": _plane(dis_c, nt),
                "dis2_p": _plane(dis_c * dis_c, nt),
                "dishp_p": _plane(dish_pi, nt),
                "dishh_p": _plane(dishh, nt),
                "ratio_p": _plane(ratio, nt),
                "gperm_p": _plane(gperm.astype(np.int32), nt),
                "idxR_p": _plane(hpos[c].astype(np.int32), nt),
                "idx1": idx1[c],
                "idx3": idx3[c],
                "W1": W1,
                "W2": W2,
                "W3": W3,
                "M1": M1,
                "M2": M2,
                "Wd0": Wd0,
                "Wc": np.ascontiguousarray(Wc),
            }
        )

    meta = dict(
        n_cores=n_cores,
        nloc=nloc,
        nt=nt,
        ng=ng,
        n_feat=n_feat,
        D=D,
        ncls=ncls,
        K1=K1,
        K3=K3,
        core_of=core_of,
        loc_of=loc_of,
    )
    return in_maps, meta


# ------------------------------------------------------------- device build


def build(meta):
    n_cores = meta["n_cores"]
    nloc, nt, ng = meta["nloc"], meta["nt"], meta["ng"]
    n_feat, D, ncls = meta["n_feat"], meta["D"], meta["ncls"]
    K1, K3 = meta["K1"], meta["K3"]
    DD = 2 * D
    nfc = n_feat // P
    sk1, sk3 = sum(K1), sum(K3)
    groups = [list(range(n_cores))]

    nc = bacc.Bacc("TRN2", debug=False, num_devices=n_cores)
    shared = "Shared" if n_cores > 4 else "Local"

    xT = nc.dram_tensor("xT", [n_feat, nloc], F32, kind="ExternalInput")
    dis_p = nc.dram_tensor("dis_p", [P, nt], F32, kind="ExternalInput")
    dis2_p = nc.dram_tensor("dis2_p", [P, nt], F32, kind="ExternalInput")
    dishp_p = nc.dram_tensor("dishp_p", [P, nt], F32, kind="ExternalInput")
    dishh_p = nc.dram_tensor("dishh_p", [P, nt], F32, kind="ExternalInput")
    ratio_p = nc.dram_tensor("ratio_p", [P, nt], F32, kind="ExternalInput")
    gperm_p = nc.dram_tensor("gperm_p", [P, nt], I32, kind="ExternalInput")
    idxR_p = nc.dram_tensor("idxR_p", [P, nt], I32, kind="ExternalInput")
    idx1 = nc.dram_tensor("idx1", [P, sk1], I32, kind="ExternalInput")
    idx3 = nc.dram_tensor("idx3", [P, sk3], I32, kind="ExternalInput")
    W1 = nc.dram_tensor("W1", [n_feat, D], F32, kind="ExternalInput")
    W2 = nc.dram_tensor("W2", [D, D], F32, kind="ExternalInput")
    W3 = nc.dram_tensor("W3", [D, D], F32, kind="ExternalInput")
    M1 = nc.dram_tensor("M1", [D, D], F32, kind="ExternalInput")
    M2 = nc.dram_tensor("M2", [D, D], F32, kind="ExternalInput")
    Wd0 = nc.dram_tensor("Wd0", [D, D], F32, kind="ExternalInput")
    Wc = nc.dram_tensor("Wc", [D, ncls], F32, kind="ExternalInput")
    out = nc.dram_tensor("out", [nloc, ncls + 2], F32, kind="ExternalOutput")

    xw_s = nc.dram_tensor("xw_s", [nloc, D], TDT, kind="Internal")
    XW = nc.dram_tensor("XW", [ng, D], TDT, kind="Internal", addr_space=shared)
    t1_s = nc.dram_tensor("t1_s", [nloc, DD], TDT, kind="Internal")
    T1 = nc.dram_tensor("T1", [ng, DD], TDT, kind="Internal", addr_space=shared)
    zd_s = nc.dram_tensor("zd_s", [nloc, DD], TDT, kind="Internal")
    ZD = nc.dram_tensor("ZD", [ng, DD], TDT, kind="Internal", addr_space=shared)
    e1_s = nc.dram_tensor("e1_s", [nloc, D], F32, kind="Internal")
    e1h_s = nc.dram_tensor("e1h_s", [nloc, DD], TDT, kind="Internal")
    e1d_s = nc.dram_tensor("e1d_s", [nloc, D], TDT, kind="Internal")
    E1H = nc.dram_tensor("E1H", [ng, DD], TDT, kind="Internal", addr_space=shared)
    E1D = nc.dram_tensor("E1D", [ng, D], TDT, kind="Internal", addr_space=shared)
    E2h = nc.dram_tensor("E2h", [nloc, DD], F32, kind="Internal")
    TV = nc.dram_tensor("TV", [nloc, D], F32, kind="Internal")

    with tile.TileContext(nc) as tc:
        with (
            tc.tile_pool(name="const", bufs=1) as constp,
            tc.tile_pool(name="gath", bufs=3) as gathp,
            tc.tile_pool(name="work", bufs=3) as workp,
            tc.tile_pool(name="outp", bufs=3) as outp,
            tc.tile_pool(name="psum", bufs=2, space="PSUM") as psp,
        ):
            ident = constp.tile([P, P], F32)
            make_identity(nc, ident[:])

            # resident planes + indices
            def res(t_dram, w, dt=F32, name=None):
                tl = constp.tile([P, w], dt, name=name)
                nc.sync.dma_start(tl[:], t_dram.ap())
                return tl

            disq = res(dis_p, nt, name="disq")
            dis2q = res(dis2_p, nt, name="dis2q")
            dishpq = res(dishp_p, nt, name="dishpq")
            dishhq = res(dishh_p, nt, name="dishhq")
            ratioq = res(ratio_p, nt, name="ratioq")
            gpermq = res(gperm_p, nt, I32, name="gpermq")
            idxRq = res(idxR_p, nt, I32, name="idxRq")
            idx1q = res(idx1, sk1, I32, name="idx1q")
            idx3q = res(idx3, sk3, I32, name="idx3q")

            w1t = [
                constp.tile([P, D], F32, name=f"w1t_{i}") for i in range(nfc)
            ]
            for i in range(nfc):
                nc.sync.dma_start(w1t[i][:], W1.ap()[i * P : (i + 1) * P])
            w2t = res(W2, D, name="w2t")
            w3t = res(W3, D, name="w3t")
            m1t = res(M1, D, name="m1t")
            m2t = res(M2, D, name="m2t")
            wdt = res(Wd0, D, name="wdt")
            wct = res(Wc, ncls, name="wct")

            def rows(t):
                return slice(t * P, (t + 1) * P)

            def col(plane, t):
                return plane[:, t : t + 1]

            # ---- S0: xW1' shard
            for t in range(nt):
                ps = psp.tile([P, D], F32, tag="mm")
                for i in range(nfc):
                    xt = workp.tile([P, P], F32, tag="xt")
                    nc.sync.dma_start(xt[:], xT.ap()[i * P : (i + 1) * P, rows(t)])
                    nc.tensor.matmul(
                        out=ps[:],
                        lhsT=xt[:],
                        rhs=w1t[i][:],
                        start=(i == 0),
                        stop=(i == nfc - 1),
                    )
                o = outp.tile([P, D], TDT, tag="s0")
                nc.vector.tensor_scalar_mul(o[:], ps[:], col(disq, t))
                nc.sync.dma_start(xw_s.ap()[rows(t)], o[:])

            nc.gpsimd.collective_compute(
                "AllGather",
                ALU.bypass,
                replica_groups=groups,
                ins=[xw_s[:].opt()],
                outs=[XW[:].opt()],
            )

            # ---- S2: T1 shard
            for t in range(nt):
                g = gathp.tile([P, D], TDT, tag="g2")
                nc.gpsimd.indirect_dma_start(
                    out=g[:],
                    out_offset=None,
                    in_=XW.ap(),
                    in_offset=bass.IndirectOffsetOnAxis(ap=col(gpermq, t), axis=0),
                )
                o = outp.tile([P, D], TDT, tag="s2")
                nc.vector.tensor_scalar_mul(o[:], g[:], col(ratioq, t))
                nc.sync.dma_start(t1_s.ap()[rows(t), D:DD], o[:])
                l = workp.tile([P, D], TDT, tag="s2l")
                nc.sync.dma_start(l[:], xw_s.ap()[rows(t)])
                nc.sync.dma_start(t1_s.ap()[rows(t), 0:D], l[:])

            nc.gpsimd.collective_compute(
                "AllGather",
                ALU.bypass,
                replica_groups=groups,
                ins=[t1_s[:].opt()],
                outs=[T1[:].opt()],
            )

            # ---- generic ELL gather: returns aggregated [P, width] tile.
            # Slots fold pairwise in bf16 (DVE 2x mode, contiguous slabs);
            # odd leftovers + optional direct-DMA'd self term combine in f32.
            def ell_tile(table, width, K, koff, idxq, self_dram=None, t=None):
                g = gathp.tile([P, max(K, 1) * width], TDT, tag="ge")
                for k in range(K):
                    nc.gpsimd.indirect_dma_start(
                        out=g[:, k * width : (k + 1) * width],
                        out_offset=None,
                        in_=table.ap(),
                        in_offset=bass.IndirectOffsetOnAxis(
                            ap=idxq[:, koff + k : koff + k + 1], axis=0
                        ),
                    )
                lt = None
                if self_dram is not None:
                    lt = workp.tile([P, width], TDT, tag="selfl")
                    nc.sync.dma_start(lt[:], self_dram.ap()[rows(t)])
                k = K
                extras = []
                while k > 1:
                    h = k // 2
                    nc.vector.tensor_tensor(
                        out=g[:, 0 : h * width],
                        in0=g[:, 0 : h * width],
                        in1=g[:, h * width : 2 * h * width],
                        op=ALU.add,
                    )
                    if k & 1:
                        extras.append(2 * h)
                    k = h
                terms = []
                if K >= 1:
                    terms.append(g[:, 0:width])
                terms += [g[:, e * width : (e + 1) * width] for e in extras]
                if lt is not None:
                    terms.append(lt[:])
                s = workp.tile([P, width], F32, tag="se")
                if len(terms) == 1:
                    nc.vector.tensor_copy(s[:], terms[0])
                else:
                    nc.vector.tensor_tensor(
                        out=s[:], in0=terms[0], in1=terms[1], op=ALU.add
                    )
                    for x in terms[2:]:
                        nc.vector.tensor_tensor(out=s[:], in0=s[:], in1=x, op=ALU.add)
                return s

            # ---- G1: zd = relu(dis2 * sum) -> zd_s
            koff = 0
            for t in range(nt):
                s = ell_tile(T1, DD, K1[t], koff, idx1q, self_dram=t1_s, t=t)
                koff += K1[t]
                o = outp.tile([P, DD], TDT, tag="ze")
                nc.vector.tensor_scalar(
                    o[:], s[:], col(dis2q, t), 0.0, ALU.mult, ALU.max
                )
                nc.sync.dma_start(zd_s.ap()[rows(t)], o[:])

            nc.gpsimd.collective_compute(
                "AllGather",
                ALU.bypass,
                replica_groups=groups,
                ins=[zd_s[:].opt()],
                outs=[ZD[:].opt()],
            )

            # ---- G2: S @ W2, three shipped variants
            koff = 0
            for t in range(nt):
                s = ell_tile(ZD, DD, K1[t], koff, idx1q, self_dram=zd_s, t=t)
                koff += K1[t]
                e1h = outp.tile([P, DD], TDT, tag="e1h")
                e1d = outp.tile([P, D], TDT, tag="e1d")
                e1p = outp.tile([P, D], F32, tag="e1p")
                for h in range(2):
                    tp = psp.tile([P, P], F32, tag="t", bufs=3)
                    nc.tensor.transpose(
                        out=tp[:], in_=s[:, h * D : (h + 1) * D], identity=ident[:]
                    )
                    tps = workp.tile([P, P], F32, tag="tps")
                    nc.vector.tensor_copy(tps[:], tp[:])
                    mm = psp.tile([P, D], F32, tag="m", bufs=3)
                    nc.tensor.matmul(
                        out=mm[:], lhsT=tps[:], rhs=w2t[:], start=True, stop=True
                    )
                    # e1 = relu(dis * mm)
                    eh = workp.tile([P, D], F32, tag="eh")
                    nc.vector.tensor_scalar(
                        eh[:], mm[:], col(disq, t), 0.0, ALU.mult, ALU.max
                    )
                    nc.vector.tensor_scalar_mul(
                        e1h[:, h * D : (h + 1) * D], eh[:], col(dishpq, t)
                    )
                    if h == 0:
                        nc.vector.tensor_copy(e1p[:], eh[:])
                        nc.vector.tensor_scalar_mul(e1d[:], eh[:], col(disq, t))
                nc.sync.dma_start(e1_s.ap()[rows(t)], e1p[:])
                nc.sync.dma_start(e1h_s.ap()[rows(t)], e1h[:])
                nc.sync.dma_start(e1d_s.ap()[rows(t)], e1d[:])

            # E1D ships first: G4's gathers depend only on it, and they hide
            # the bigger E1H AllGather that G3 needs.
            nc.gpsimd.collective_compute(
                "AllGather",
                ALU.bypass,
                replica_groups=groups,
                ins=[e1d_s[:].opt()],
                outs=[E1D[:].opt()],
            )
            nc.gpsimd.collective_compute(
                "AllGather",
                ALU.bypass,
                replica_groups=groups,
                ins=[e1h_s[:].opt()],
                outs=[E1H[:].opt()],
            )

            # ---- G4: cls = (dis * sum) @ Wc -> out[:, :ncls]
            koff = 0
            for t in range(nt):
                s = ell_tile(E1D, D, K1[t], koff, idx1q, self_dram=e1d_s, t=t)
                koff += K1[t]
                sc_ = workp.tile([P, D], F32, tag="c_s")
                nc.vector.tensor_scalar_mul(sc_[:], s[:], col(disq, t))
                tp = psp.tile([P, P], F32, tag="t", bufs=3)
                nc.tensor.transpose(out=tp[:], in_=sc_[:], identity=ident[:])
                tps = workp.tile([P, P], F32, tag="tps")
                nc.vector.tensor_copy(tps[:], tp[:])
                mm = psp.tile([P, ncls], F32, tag="m", bufs=3)
                nc.tensor.matmul(out=mm[:], lhsT=tps[:], rhs=wct[:], start=True, stop=True)
                o = outp.tile([P, ncls], F32, tag="c_o")
                nc.vector.tensor_copy(o[:], mm[:])
                nc.sync.dma_start(out.ap()[rows(t), 0:ncls], o[:])

            # ---- S12: MLP + tvec (local, overlaps with G4/G3 gathers)
            for t in range(nt):
                et = workp.tile([P, D], F32, tag="ml_in")
                nc.sync.dma_start(et[:], e1_s.ap()[rows(t)])
                tp = psp.tile([P, P], F32, tag="t", bufs=3)
                nc.tensor.transpose(out=tp[:], in_=et[:], identity=ident[:])
                tps = workp.tile([P, P], F32, tag="tps")
                nc.vector.tensor_copy(tps[:], tp[:])
                mm = psp.tile([P, D], F32, tag="m", bufs=3)
                nc.tensor.matmul(out=mm[:], lhsT=tps[:], rhs=m1t[:], start=True, stop=True)
                u = workp.tile([P, D], F32, tag="ml_u")
                nc.scalar.activation(u[:], mm[:], AF.Relu)
                tp2 = psp.tile([P, P], F32, tag="t", bufs=3)
                nc.tensor.transpose(out=tp2[:], in_=u[:], identity=ident[:])
                tps2 = workp.tile([P, P], F32, tag="tps")
                nc.vector.tensor_copy(tps2[:], tp2[:])
                mm2 = psp.tile([P, D], F32, tag="m", bufs=3)
                nc.tensor.matmul(
                    out=mm2[:], lhsT=tps2[:], rhs=m2t[:], start=True, stop=True
                )
                e3 = workp.tile([P, D], F32, tag="ml_e3")
                nc.vector.tensor_copy(e3[:], mm2[:])
                tp3 = psp.tile([P, P], F32, tag="t", bufs=3)
                nc.tensor.transpose(out=tp3[:], in_=e3[:], identity=ident[:])
                tps3 = workp.tile([P, P], F32, tag="tps")
                nc.vector.tensor_copy(tps3[:], tp3[:])
                mm3 = psp.tile([P, D], F32, tag="m", bufs=3)
                nc.tensor.matmul(
                    out=mm3[:], lhsT=tps3[:], rhs=wdt[:], start=True, stop=True
                )
                tv = outp.tile([P, D], F32, tag="ml_tv")
                nc.vector.tensor_copy(tv[:], mm3[:])
                nc.sync.dma_start(TV.ap()[rows(t)], tv[:])

            # ---- G3: embed2{,b} = dishh * (S_h @ W3) -> E2h (hop order)
            koff = 0
            for t in range(nt):
                s = ell_tile(E1H, DD, K3[t], koff, idx3q)
                koff += K3[t]
                e2 = outp.tile([P, DD], F32, tag="e2")
                for h in range(2):
                    tp = psp.tile([P, P], F32, tag="t", bufs=3)
                    nc.tensor.transpose(
                        out=tp[:], in_=s[:, h * D : (h + 1) * D], identity=ident[:]
                    )
                    tps = workp.tile([P, P], F32, tag="tps")
                    nc.vector.tensor_copy(tps[:], tp[:])
                    mm = psp.tile([P, D], F32, tag="m", bufs=3)
                    nc.tensor.matmul(
                        out=mm[:], lhsT=tps[:], rhs=w3t[:], start=True, stop=True
                    )
                    nc.vector.tensor_scalar_mul(
                        e2[:, h * D : (h + 1) * D], mm[:], col(dishhq, t)
                    )
                nc.sync.dma_start(E2h.ap()[rows(t)], e2[:])

            # ---- S11 + S13: realign + scores
            for t in range(nt):
                e2 = gathp.tile([P, DD], F32, tag="gr")
                nc.gpsimd.indirect_dma_start(
                    out=e2[:],
                    out_offset=None,
                    in_=E2h.ap(),
                    in_offset=bass.IndirectOffsetOnAxis(ap=col(idxRq, t), axis=0),
                )
                tv = workp.tile([P, D], F32, tag="sc_tv")
                nc.sync.dma_start(tv[:], TV.ap()[rows(t)])
                pr = workp.tile([P, DD], F32, tag="sc_pr")
                nc.vector.tensor_mul(pr[:, 0:D], tv[:], e2[:, 0:D])
                nc.vector.tensor_mul(pr[:, D:DD], tv[:], e2[:, D:DD])
                rs = workp.tile([P, 2], F32, tag="sc_rs")
                nc.vector.tensor_reduce(
                    out=rs[:],
                    in_=pr[:].rearrange("p (h d) -> p h d", h=2),
                    axis=mybir.AxisListType.X,
                    op=ALU.add,
                )
                sg = outp.tile([P, 2], F32, tag="sc_sg")
                nc.scalar.activation(sg[:], rs[:], AF.Sigmoid)
                nc.sync.dma_start(out.ap()[rows(t), ncls : ncls + 2], sg[:])

    nc.compile()
    return nc


def assemble(results, meta):
    n_cores = meta["n_cores"]
    N = len(meta["core_of"])
    ncls = meta["ncls"]
    out = np.empty((N, ncls + 2), np.float32)
    for c in range(n_cores):
        oc = results[c]["out"]
        m = meta["core_of"] == c
        out[m] = oc[meta["loc_of"][m]]
    return out


# ------------------------------------------------------------------ entry


_CACHE = {}


def kernel(**inputs):
    """Full-input entry point: shards across 8 NeuronCores internally.

    Expects the nn_MixModel input dict (x, edge_index, edge_index_hop, y,
    perm, W1..Wd); returns the full [N, n_cls+2] float32 output.
    """
    n_cores = 8
    in_maps, meta = prep(inputs, n_cores)
    key = (meta["nloc"], tuple(meta["K1"]), tuple(meta["K3"]))
    nc = _CACHE.get(key)
    if nc is None:
        nc = build(meta)
        _CACHE[key] = nc
    res = bass_utils.run_bass_kernel_spmd(
        nc, in_maps, core_ids=list(range(n_cores)), trace=False
    )
    return assemble(res.results, meta)



# revision 14
# speedup vs baseline: 1.0485x; 1.0256x over previous
"""8-core Trainium2 Bass kernel for nn_MixModel (GCN mix model) — v2.

Sharding: nodes dealt round-robin by in-degree rank to 8 cores; each core owns
NLOC = ceil((ceil(N/8)+1)/128)*128 local rows (>=1 zero pad row reused as the
ELL gather-pad target).

Algebra used:
 - GCN messages factorize: msg = (h*dis)[src], output scaled by dis[dst]; the
   self-loop is one extra ELL slot. Aggregation = unweighted padded-ELL
   gather+sum of pre-scaled table rows.
 - segsum and the layer matmul commute: sum((z@W*dis)[src]) =
   sum((z*dis)[src]) @ W — so cores AllGather the *scaled activations* and the
   per-layer matmul runs on the 98 aggregated dst tiles inside the gather
   pipeline (hidden under Q7 descriptor generation), not on 784 table tiles.
 - good/bad paths share edge sets -> gather concatenated 256-wide tables.
 - the permuted-input path's first-layer table is a cheap local permutation
   gather of the xW1' table (12.5k rows), not a per-edge pass.

Stages (per core):
  S0   xW1' shard = (x_sh @ W1) * dis_sh
  AG0  AllGather -> XW [NG,128]
  S2   T1 shard = [xW1'_loc | gather(XW, gperm)*ratio] ; AG1 -> T1 [NG,256]
  G1   ELL gather T1 -> zd = relu(dis^2 * sum)  (= z1*dis)      -> AG2 ZD
  G2   ELL gather ZD -> S ; e1{,b} = relu(dis * (S_h @ W2)) ;
       ship [e1*dish|e1b*dish] -> AG3a E1H ; [e1*dis] -> AG3b E1D ; e1 local
  G3   ELL gather E1H (hop order) -> embed2{,b} = dish * (S_h @ W3) -> E2h
  S12  MLP: embed3 = relu(e1@M1)@M2 ; tvec = embed3@Wd0
  S11  realign E2h to pi order ; scores = sigmoid(rowsum(tvec * e2{,b}))
  G4   ELL gather E1D -> cls = (dis*sum)@Wc -> OUT[:, :10]
"""

import numpy as np

import concourse.bacc as bacc
import concourse.bass as bass
import concourse.mybir as mybir
import concourse.tile as tile
from concourse import bass_utils
from concourse.masks import make_identity

P = 128
F32 = mybir.dt.float32
I32 = mybir.dt.int32
AF = mybir.ActivationFunctionType
ALU = mybir.AluOpType
TDT = mybir.dt.bfloat16  # transport/table dtype


# ----------------------------------------------------------------- host prep


def _ell_build(src_g, dst_core, dst_loc, self_g, n_cores, nloc, padrow):
    """Shared-K ELL: returns (K per tile, per-core int32 [P, sum(K)] arrays,
    p-major-global: element [p, koff[t]+k] = slot k of local row t*128+p)."""
    nt = nloc // P
    counts = np.zeros((n_cores, nloc), np.int64)
    np.add.at(counts, (dst_core, dst_loc), 1)
    n_self = 0 if self_g is None else 1
    cmax = counts.reshape(n_cores, nt, P).max(axis=(0, 2))
    K = (cmax + n_self).astype(np.int64)
    order = np.lexsort((dst_loc, dst_core))
    sc, sl, sg = dst_core[order], dst_loc[order], src_g[order]
    key = sc.astype(np.int64) * nloc + sl
    is_start = np.r_[True, key[1:] != key[:-1]] if len(key) else np.array([], bool)
    run_starts = np.flatnonzero(is_start)
    run_len = np.diff(np.r_[run_starts, len(key)])
    pos_in_run = np.arange(len(key)) - np.repeat(run_starts, run_len)
    koff = np.r_[0, np.cumsum(K)]
    sk = int(koff[-1])
    idx_arrs = []
    for c in range(n_cores):
        arr = np.full((P, sk), padrow[c], np.int64)
        m = sc == c
        loc, pos, gidx = sl[m], pos_in_run[m], sg[m]
        t = loc // P
        p = loc % P
        arr[p, koff[t] + pos + n_self] = gidx
        if n_self:
            allt = np.arange(nloc) // P
            allp = np.arange(nloc) % P
            arr[allp, koff[allt]] = self_g[c]
        idx_arrs.append(arr.astype(np.int32))
    return K.tolist(), idx_arrs


def _plane(vals_loc, nt):
    """[nloc] local-row vector -> [P, nt] plane (local row t*128+p -> [p, t])."""
    return np.ascontiguousarray(vals_loc.reshape(nt, P).T)


def prep(inputs, n_cores=8):
    x = np.asarray(inputs["x"], np.float32)
    ei = np.asarray(inputs["edge_index"], np.int64)
    eih = np.asarray(inputs["edge_index_hop"], np.int64)
    perm = np.asarray(inputs["perm"], np.int64)
    W1 = np.asarray(inputs["W1"], np.float32)
    W2 = np.asarray(inputs["W2"], np.float32)
    W3 = np.asarray(inputs["W3"], np.float32)
    M1 = np.asarray(inputs["M1"], np.float32)
    M2 = np.asarray(inputs["M2"], np.float32)
    Wc = np.asarray(inputs["Wc"], np.float32)
    Wd0 = np.asarray(inputs["Wd"], np.float32)[0]
    for bname in ("b1", "b2", "b3", "mb1", "mb2", "bc"):
        assert np.abs(np.asarray(inputs[bname])).max() == 0.0, (
            f"nonzero bias {bname} not supported by this kernel build"
        )

    N, n_feat = x.shape
    D = W1.shape[1]
    ncls = Wc.shape[1]
    max_real = -(-N // n_cores)
    nloc = -(-(max_real + 1) // P) * P
    nt = nloc // P
    ng = n_cores * nloc

    deg = np.bincount(ei[1], minlength=N).astype(np.float32) + 1.0
    degh = np.bincount(eih[1], minlength=N).astype(np.float32) + 1.0
    dis = 1.0 / np.sqrt(deg)
    dish = 1.0 / np.sqrt(degh)

    order = np.argsort(-deg, kind="stable")
    core_of = np.empty(N, np.int64)
    loc_of = np.empty(N, np.int64)
    core_of[order] = np.arange(N) % n_cores
    loc_of[order] = np.arange(N) // n_cores
    gl = core_of * nloc + loc_of
    padrow = [c * nloc + nloc - 1 for c in range(n_cores)]

    nat = np.full((n_cores, nloc), -1, np.int64)
    nat[core_of, loc_of] = np.arange(N)

    # hop order: per-core resort by hop degree desc (pads last)
    hkey = np.where(nat >= 0, -degh[np.maximum(nat, 0)], 1.0)
    hord = np.argsort(hkey, axis=1, kind="stable")
    hpos = np.argsort(hord, axis=1)

    selfg_pi = np.where(
        nat >= 0,
        np.arange(n_cores)[:, None] * nloc + np.arange(nloc)[None, :],
        np.array(padrow)[:, None],
    )
    # graph-g passes add the self term via a direct DMA of the local shard
    # (saves one ELL slot per tile on 3 passes); hop pass keeps self in ELL
    # (hop order makes the local slice non-contiguous).
    K1, idx1 = _ell_build(
        gl[ei[0]], core_of[ei[1]], loc_of[ei[1]], None, n_cores, nloc, padrow
    )
    selfg_h = np.take_along_axis(selfg_pi, hord, axis=1)
    K3, idx3 = _ell_build(
        gl[eih[0]],
        core_of[eih[1]],
        hpos[core_of[eih[1]], loc_of[eih[1]]],
        selfg_h,
        n_cores,
        nloc,
        padrow,
    )

    in_maps = []
    for c in range(n_cores):
        natc = nat[c]
        real = natc >= 0
        xs = np.zeros((nloc, n_feat), np.float32)
        xs[real] = x[natc[real]]
        dis_c = np.ones(nloc, np.float32)
        dis_c[real] = dis[natc[real]]
        dish_pi = np.ones(nloc, np.float32)
        dish_pi[real] = dish[natc[real]]
        dishh = np.ones(nloc, np.float32)
        hnat = natc[hord[c]]
        hreal = hnat >= 0
        dishh[hreal] = dish[hnat[hreal]]
        gperm = np.full(nloc, padrow[c], np.int64)
        ratio = np.ones(nloc, np.float32)
        pv = perm[natc[real]]
        gperm[real] = gl[pv]
        ratio[real] = dis[natc[real]] / dis[pv]
        in_maps.append(
            {
                "xT": np.ascontiguousarray(xs.T),
                "dis_p": _plane(dis_c, nt),
                "dis2_p": _plane(dis_c * dis_c, nt),
                "dishp_p": _plane(dish_pi, nt),
                "dishh_p": _plane(dishh, nt),
                "ratio_p": _plane(ratio, nt),
                "gperm_p": _plane(gperm.astype(np.int32), nt),
                "idxR_p": _plane(hpos[c].astype(np.int32), nt),
                "idx1": idx1[c],
                "idx3": idx3[c],
                "W1": W1,
                "W2": W2,
                "W3": W3,
                "M1": M1,
                "M2": M2,
                "Wd0": Wd0,
                "Wc": np.ascontiguousarray(Wc),
            }
        )

    meta = dict(
        n_cores=n_cores,
        nloc=nloc,
        nt=nt,
        ng=ng,
        n_feat=n_feat,
        D=D,
        ncls=ncls,
        K1=K1,
        K3=K3,
        core_of=core_of,
        loc_of=loc_of,
    )
    return in_maps, meta


# ------------------------------------------------------------- device build


def build(meta):
    n_cores = meta["n_cores"]
    nloc, nt, ng = meta["nloc"], meta["nt"], meta["ng"]
    n_feat, D, ncls = meta["n_feat"], meta["D"], meta["ncls"]
    K1, K3 = meta["K1"], meta["K3"]
    DD = 2 * D
    nfc = n_feat // P
    sk1, sk3 = sum(K1), sum(K3)
    groups = [list(range(n_cores))]

    nc = bacc.Bacc("TRN2", debug=False, num_devices=n_cores)
    shared = "Shared" if n_cores > 4 else "Local"

    xT = nc.dram_tensor("xT", [n_feat, nloc], F32, kind="ExternalInput")
    dis_p = nc.dram_tensor("dis_p", [P, nt], F32, kind="ExternalInput")
    dis2_p = nc.dram_tensor("dis2_p", [P, nt], F32, kind="ExternalInput")
    dishp_p = nc.dram_tensor("dishp_p", [P, nt], F32, kind="ExternalInput")
    dishh_p = nc.dram_tensor("dishh_p", [P, nt], F32, kind="ExternalInput")
    ratio_p = nc.dram_tensor("ratio_p", [P, nt], F32, kind="ExternalInput")
    gperm_p = nc.dram_tensor("gperm_p", [P, nt], I32, kind="ExternalInput")
    idxR_p = nc.dram_tensor("idxR_p", [P, nt], I32, kind="ExternalInput")
    idx1 = nc.dram_tensor("idx1", [P, sk1], I32, kind="ExternalInput")
    idx3 = nc.dram_tensor("idx3", [P, sk3], I32, kind="ExternalInput")
    W1 = nc.dram_tensor("W1", [n_feat, D], F32, kind="ExternalInput")
    W2 = nc.dram_tensor("W2", [D, D], F32, kind="ExternalInput")
    W3 = nc.dram_tensor("W3", [D, D], F32, kind="ExternalInput")
    M1 = nc.dram_tensor("M1", [D, D], F32, kind="ExternalInput")
    M2 = nc.dram_tensor("M2", [D, D], F32, kind="ExternalInput")
    Wd0 = nc.dram_tensor("Wd0", [D, D], F32, kind="ExternalInput")
    Wc = nc.dram_tensor("Wc", [D, ncls], F32, kind="ExternalInput")
    out = nc.dram_tensor("out", [nloc, ncls + 2], F32, kind="ExternalOutput")

    xw_s = nc.dram_tensor("xw_s", [nloc, D], TDT, kind="Internal")
    XW = nc.dram_tensor("XW", [ng, D], TDT, kind="Internal", addr_space=shared)
    t1_s = nc.dram_tensor("t1_s", [nloc, DD], TDT, kind="Internal")
    T1 = nc.dram_tensor("T1", [ng, DD], TDT, kind="Internal", addr_space=shared)
    zd_s = nc.dram_tensor("zd_s", [nloc, DD], TDT, kind="Internal")
    ZD = nc.dram_tensor("ZD", [ng, DD], TDT, kind="Internal", addr_space=shared)
    e1h_s = nc.dram_tensor("e1h_s", [nloc, DD], TDT, kind="Internal")
    e1d_s = nc.dram_tensor("e1d_s", [nloc, D], TDT, kind="Internal")
    E1H = nc.dram_tensor("E1H", [ng, DD], TDT, kind="Internal", addr_space=shared)
    E1D = nc.dram_tensor("E1D", [ng, D], TDT, kind="Internal", addr_space=shared)
    E2h = nc.dram_tensor("E2h", [nloc, DD], F32, kind="Internal")
    TV = nc.dram_tensor("TV", [nloc, D], F32, kind="Internal")

    with tile.TileContext(nc) as tc:
        with (
            tc.tile_pool(name="const", bufs=1) as constp,
            tc.tile_pool(name="gath", bufs=3) as gathp,
            tc.tile_pool(name="work", bufs=3) as workp,
            tc.tile_pool(name="outp", bufs=3) as outp,
            tc.tile_pool(name="psum", bufs=2, space="PSUM") as psp,
        ):
            ident = constp.tile([P, P], F32)
            make_identity(nc, ident[:])

            # resident planes + indices
            def res(t_dram, w, dt=F32, name=None):
                tl = constp.tile([P, w], dt, name=name)
                nc.sync.dma_start(tl[:], t_dram.ap())
                return tl

            disq = res(dis_p, nt, name="disq")
            dis2q = res(dis2_p, nt, name="dis2q")
            dishpq = res(dishp_p, nt, name="dishpq")
            dishhq = res(dishh_p, nt, name="dishhq")
            ratioq = res(ratio_p, nt, name="ratioq")
            gpermq = res(gperm_p, nt, I32, name="gpermq")
            idxRq = res(idxR_p, nt, I32, name="idxRq")
            idx1q = res(idx1, sk1, I32, name="idx1q")
            idx3q = res(idx3, sk3, I32, name="idx3q")

            w1t = [
                constp.tile([P, D], F32, name=f"w1t_{i}") for i in range(nfc)
            ]
            for i in range(nfc):
                nc.sync.dma_start(w1t[i][:], W1.ap()[i * P : (i + 1) * P])
            w2t = res(W2, D, name="w2t")
            w3t = res(W3, D, name="w3t")
            m1t = res(M1, D, name="m1t")
            m2t = res(M2, D, name="m2t")
            wdt = res(Wd0, D, name="wdt")
            wct = res(Wc, ncls, name="wct")

            def rows(t):
                return slice(t * P, (t + 1) * P)

            def col(plane, t):
                return plane[:, t : t + 1]

            # ---- S0: xW1' shard
            for t in range(nt):
                ps = psp.tile([P, D], F32, tag="mm")
                for i in range(nfc):
                    xt = workp.tile([P, P], F32, tag="xt")
                    nc.sync.dma_start(xt[:], xT.ap()[i * P : (i + 1) * P, rows(t)])
                    nc.tensor.matmul(
                        out=ps[:],
                        lhsT=xt[:],
                        rhs=w1t[i][:],
                        start=(i == 0),
                        stop=(i == nfc - 1),
                    )
                o = outp.tile([P, D], TDT, tag="s0")
                nc.vector.tensor_scalar_mul(o[:], ps[:], col(disq, t))
                nc.sync.dma_start(xw_s.ap()[rows(t)], o[:])

            nc.gpsimd.collective_compute(
                "AllGather",
                ALU.bypass,
                replica_groups=groups,
                ins=[xw_s[:].opt()],
                outs=[XW[:].opt()],
            )

            # ---- S2: T1 shard
            for t in range(nt):
                g = gathp.tile([P, D], TDT, tag="g2")
                nc.gpsimd.indirect_dma_start(
                    out=g[:],
                    out_offset=None,
                    in_=XW.ap(),
                    in_offset=bass.IndirectOffsetOnAxis(ap=col(gpermq, t), axis=0),
                )
                o = outp.tile([P, D], TDT, tag="s2")
                nc.vector.tensor_scalar_mul(o[:], g[:], col(ratioq, t))
                nc.sync.dma_start(t1_s.ap()[rows(t), D:DD], o[:])
                l = workp.tile([P, D], TDT, tag="s2l")
                nc.sync.dma_start(l[:], xw_s.ap()[rows(t)])
                nc.sync.dma_start(t1_s.ap()[rows(t), 0:D], l[:])

            nc.gpsimd.collective_compute(
                "AllGather",
                ALU.bypass,
                replica_groups=groups,
                ins=[t1_s[:].opt()],
                outs=[T1[:].opt()],
            )

            # ---- generic ELL gather: returns aggregated [P, width] tile.
            # Slots fold pairwise in bf16 (DVE 2x mode, contiguous slabs);
            # odd leftovers + optional direct-DMA'd self term combine in f32.
            def ell_tile(table, width, K, koff, idxq, self_dram=None, t=None):
                g = gathp.tile([P, max(K, 1) * width], TDT, tag="ge")
                for k in range(K):
                    nc.gpsimd.indirect_dma_start(
                        out=g[:, k * width : (k + 1) * width],
                        out_offset=None,
                        in_=table.ap(),
                        in_offset=bass.IndirectOffsetOnAxis(
                            ap=idxq[:, koff + k : koff + k + 1], axis=0
                        ),
                    )
                lt = None
                if self_dram is not None:
                    lt = workp.tile([P, width], TDT, tag="selfl")
                    nc.sync.dma_start(lt[:], self_dram.ap()[rows(t)])
                k = K
                extras = []
                while k > 1:
                    h = k // 2
                    nc.vector.tensor_tensor(
                        out=g[:, 0 : h * width],
                        in0=g[:, 0 : h * width],
                        in1=g[:, h * width : 2 * h * width],
                        op=ALU.add,
                    )
                    if k & 1:
                        extras.append(2 * h)
                    k = h
                terms = []
                if K >= 1:
                    terms.append(g[:, 0:width])
                terms += [g[:, e * width : (e + 1) * width] for e in extras]
                if lt is not None:
                    terms.append(lt[:])
                s = workp.tile([P, width], F32, tag="se")
                if len(terms) == 1:
                    nc.vector.tensor_copy(s[:], terms[0])
                else:
                    nc.vector.tensor_tensor(
                        out=s[:], in0=terms[0], in1=terms[1], op=ALU.add
                    )
                    for x in terms[2:]:
                        nc.vector.tensor_tensor(out=s[:], in0=s[:], in1=x, op=ALU.add)
                return s

            # ---- G1: zd = relu(dis2 * sum) -> zd_s
            koff = 0
            for t in range(nt):
                s = ell_tile(T1, DD, K1[t], koff, idx1q, self_dram=t1_s, t=t)
                koff += K1[t]
                o = outp.tile([P, DD], TDT, tag="ze")
                nc.vector.tensor_scalar(
                    o[:], s[:], col(dis2q, t), 0.0, ALU.mult, ALU.max
                )
                nc.sync.dma_start(zd_s.ap()[rows(t)], o[:])

            nc.gpsimd.collective_compute(
                "AllGather",
                ALU.bypass,
                replica_groups=groups,
                ins=[zd_s[:].opt()],
                outs=[ZD[:].opt()],
            )

            # ---- G2: S @ W2, three shipped variants
            koff = 0
            for t in range(nt):
                s = ell_tile(ZD, DD, K1[t], koff, idx1q, self_dram=zd_s, t=t)
                koff += K1[t]
                e1h = outp.tile([P, DD], TDT, tag="e1h")
                e1d = outp.tile([P, D], TDT, tag="e1d")
                e1p = outp.tile([P, D], F32, tag="e1p")
                for h in range(2):
                    tp = psp.tile([P, P], F32, tag="t", bufs=3)
                    nc.tensor.transpose(
                        out=tp[:], in_=s[:, h * D : (h + 1) * D], identity=ident[:]
                    )
                    tps = workp.tile([P, P], F32, tag="tps")
                    nc.vector.tensor_copy(tps[:], tp[:])
                    mm = psp.tile([P, D], F32, tag="m", bufs=3)
                    nc.tensor.matmul(
                        out=mm[:], lhsT=tps[:], rhs=w2t[:], start=True, stop=True
                    )
                    # e1 = relu(dis * mm)
                    eh = workp.tile([P, D], F32, tag="eh")
                    nc.vector.tensor_scalar(
                        eh[:], mm[:], col(disq, t), 0.0, ALU.mult, ALU.max
                    )
                    nc.vector.tensor_scalar_mul(
                        e1h[:, h * D : (h + 1) * D], eh[:], col(dishpq, t)
                    )
                    if h == 0:
                        nc.vector.tensor_copy(e1p[:], eh[:])
                        nc.vector.tensor_scalar_mul(e1d[:], eh[:], col(disq, t))
                nc.sync.dma_start(e1h_s.ap()[rows(t)], e1h[:])
                nc.sync.dma_start(e1d_s.ap()[rows(t)], e1d[:])
                # S12 folded in: MLP + tvec on the in-SBUF e1p tile; its
                # Tensor/Vector ops hide under this loop's gather issue.
                tpa = psp.tile([P, P], F32, tag="t", bufs=3)
                nc.tensor.transpose(out=tpa[:], in_=e1p[:], identity=ident[:])
                tpsa = workp.tile([P, P], F32, tag="tps")
                nc.vector.tensor_copy(tpsa[:], tpa[:])
                mma = psp.tile([P, D], F32, tag="m", bufs=3)
                nc.tensor.matmul(out=mma[:], lhsT=tpsa[:], rhs=m1t[:], start=True, stop=True)
                u = workp.tile([P, D], F32, tag="ml_u")
                nc.scalar.activation(u[:], mma[:], AF.Relu)
                tpb = psp.tile([P, P], F32, tag="t", bufs=3)
                nc.tensor.transpose(out=tpb[:], in_=u[:], identity=ident[:])
                tpsb = workp.tile([P, P], F32, tag="tps")
                nc.vector.tensor_copy(tpsb[:], tpb[:])
                mmb = psp.tile([P, D], F32, tag="m", bufs=3)
                nc.tensor.matmul(out=mmb[:], lhsT=tpsb[:], rhs=m2t[:], start=True, stop=True)
                e3 = workp.tile([P, D], F32, tag="ml_e3")
                nc.vector.tensor_copy(e3[:], mmb[:])
                tpc = psp.tile([P, P], F32, tag="t", bufs=3)
                nc.tensor.transpose(out=tpc[:], in_=e3[:], identity=ident[:])
                tpsc = workp.tile([P, P], F32, tag="tps")
                nc.vector.tensor_copy(tpsc[:], tpc[:])
                mmc = psp.tile([P, D], F32, tag="m", bufs=3)
                nc.tensor.matmul(out=mmc[:], lhsT=tpsc[:], rhs=wdt[:], start=True, stop=True)
                tv = outp.tile([P, D], F32, tag="ml_tv")
                nc.vector.tensor_copy(tv[:], mmc[:])
                nc.sync.dma_start(TV.ap()[rows(t)], tv[:])

            # E1D ships first: G4's gathers depend only on it, and they hide
            # the bigger E1H AllGather that G3 needs.
            nc.gpsimd.collective_compute(
                "AllGather",
                ALU.bypass,
                replica_groups=groups,
                ins=[e1d_s[:].opt()],
                outs=[E1D[:].opt()],
            )
            nc.gpsimd.collective_compute(
                "AllGather",
                ALU.bypass,
                replica_groups=groups,
                ins=[e1h_s[:].opt()],
                outs=[E1H[:].opt()],
            )

            # ---- G4: cls = (dis * sum) @ Wc -> out[:, :ncls]
            koff = 0
            for t in range(nt):
                s = ell_tile(E1D, D, K1[t], koff, idx1q, self_dram=e1d_s, t=t)
                koff += K1[t]
                sc_ = workp.tile([P, D], F32, tag="c_s")
                nc.vector.tensor_scalar_mul(sc_[:], s[:], col(disq, t))
                tp = psp.tile([P, P], F32, tag="t", bufs=3)
                nc.tensor.transpose(out=tp[:], in_=sc_[:], identity=ident[:])
                tps = workp.tile([P, P], F32, tag="tps")
                nc.vector.tensor_copy(tps[:], tp[:])
                mm = psp.tile([P, ncls], F32, tag="m", bufs=3)
                nc.tensor.matmul(out=mm[:], lhsT=tps[:], rhs=wct[:], start=True, stop=True)
                o = outp.tile([P, ncls], F32, tag="c_o")
                nc.vector.tensor_copy(o[:], mm[:])
                nc.sync.dma_start(out.ap()[rows(t), 0:ncls], o[:])

            # ---- G3: embed2{,b} = dishh * (S_h @ W3) -> E2h (hop order)
            koff = 0
            for t in range(nt):
                s = ell_tile(E1H, DD, K3[t], koff, idx3q)
                koff += K3[t]
                e2 = outp.tile([P, DD], F32, tag="e2")
                for h in range(2):
                    tp = psp.tile([P, P], F32, tag="t", bufs=3)
                    nc.tensor.transpose(
                        out=tp[:], in_=s[:, h * D : (h + 1) * D], identity=ident[:]
                    )
                    tps = workp.tile([P, P], F32, tag="tps")
                    nc.vector.tensor_copy(tps[:], tp[:])
                    mm = psp.tile([P, D], F32, tag="m", bufs=3)
                    nc.tensor.matmul(
                        out=mm[:], lhsT=tps[:], rhs=w3t[:], start=True, stop=True
                    )
                    nc.vector.tensor_scalar_mul(
                        e2[:, h * D : (h + 1) * D], mm[:], col(dishhq, t)
                    )
                nc.sync.dma_start(E2h.ap()[rows(t)], e2[:])

            # ---- S11 + S13: realign + scores
            for t in range(nt):
                e2 = gathp.tile([P, DD], F32, tag="gr")
                nc.gpsimd.indirect_dma_start(
                    out=e2[:],
                    out_offset=None,
                    in_=E2h.ap(),
                    in_offset=bass.IndirectOffsetOnAxis(ap=col(idxRq, t), axis=0),
                )
                tv = workp.tile([P, D], F32, tag="sc_tv")
                nc.sync.dma_start(tv[:], TV.ap()[rows(t)])
                pr = workp.tile([P, DD], F32, tag="sc_pr")
                nc.vector.tensor_mul(pr[:, 0:D], tv[:], e2[:, 0:D])
                nc.vector.tensor_mul(pr[:, D:DD], tv[:], e2[:, D:DD])
                rs = workp.tile([P, 2], F32, tag="sc_rs")
                nc.vector.tensor_reduce(
                    out=rs[:],
                    in_=pr[:].rearrange("p (h d) -> p h d", h=2),
                    axis=mybir.AxisListType.X,
                    op=ALU.add,
                )
                sg = outp.tile([P, 2], F32, tag="sc_sg")
                nc.scalar.activation(sg[:], rs[:], AF.Sigmoid)
                nc.sync.dma_start(out.ap()[rows(t), ncls : ncls + 2], sg[:])

    nc.compile()
    return nc


def assemble(results, meta):
    n_cores = meta["n_cores"]
    N = len(meta["core_of"])
    ncls = meta["ncls"]
    out = np.empty((N, ncls + 2), np.float32)
    for c in range(n_cores):
        oc = results[c]["out"]
        m = meta["core_of"] == c
        out[m] = oc[meta["loc_of"][m]]
    return out


# ------------------------------------------------------------------ entry


_CACHE = {}


def kernel(**inputs):
    """Full-input entry point: shards across 8 NeuronCores internally.

    Expects the nn_MixModel input dict (x, edge_index, edge_index_hop, y,
    perm, W1..Wd); returns the full [N, n_cls+2] float32 output.
    """
    n_cores = 8
    in_maps, meta = prep(inputs, n_cores)
    key = (meta["nloc"], tuple(meta["K1"]), tuple(meta["K3"]))
    nc = _CACHE.get(key)
    if nc is None:
        nc = build(meta)
        _CACHE[key] = nc
    res = bass_utils.run_bass_kernel_spmd(
        nc, in_maps, core_ids=list(range(n_cores)), trace=False
    )
    return assemble(res.results, meta)



# revision 16
# speedup vs baseline: 1.0633x; 1.0141x over previous
"""8-core Trainium2 Bass kernel for nn_MixModel (GCN mix model) — v2.

Sharding: nodes dealt round-robin by in-degree rank to 8 cores; each core owns
NLOC = ceil((ceil(N/8)+1)/128)*128 local rows (>=1 zero pad row reused as the
ELL gather-pad target).

Algebra used:
 - GCN messages factorize: msg = (h*dis)[src], output scaled by dis[dst]; the
   self-loop is one extra ELL slot. Aggregation = unweighted padded-ELL
   gather+sum of pre-scaled table rows.
 - segsum and the layer matmul commute: sum((z@W*dis)[src]) =
   sum((z*dis)[src]) @ W — so cores AllGather the *scaled activations* and the
   per-layer matmul runs on the 98 aggregated dst tiles inside the gather
   pipeline (hidden under Q7 descriptor generation), not on 784 table tiles.
 - good/bad paths share edge sets -> gather concatenated 256-wide tables.
 - the permuted-input path's first-layer table is a cheap local permutation
   gather of the xW1' table (12.5k rows), not a per-edge pass.

Stages (per core):
  S0   xW1' shard = (x_sh @ W1) * dis_sh
  AG0  AllGather -> XW [NG,128]
  S2   T1 shard = [xW1'_loc | gather(XW, gperm)*ratio] ; AG1 -> T1 [NG,256]
  G1   ELL gather T1 -> zd = relu(dis^2 * sum)  (= z1*dis)      -> AG2 ZD
  G2   ELL gather ZD -> S ; e1{,b} = relu(dis * (S_h @ W2)) ;
       ship [e1*dish|e1b*dish] -> AG3a E1H ; [e1*dis] -> AG3b E1D ; e1 local
  G3   ELL gather E1H (hop order) -> embed2{,b} = dish * (S_h @ W3) -> E2h
  S12  MLP: embed3 = relu(e1@M1)@M2 ; tvec = embed3@Wd0
  S11  realign E2h to pi order ; scores = sigmoid(rowsum(tvec * e2{,b}))
  G4   ELL gather E1D -> cls = (dis*sum)@Wc -> OUT[:, :10]
"""

import ml_dtypes
import numpy as np

import concourse.bacc as bacc
import concourse.bass as bass
import concourse.mybir as mybir
import concourse.tile as tile
from concourse import bass_utils
from concourse.masks import make_identity

P = 128
F32 = mybir.dt.float32
I32 = mybir.dt.int32
AF = mybir.ActivationFunctionType
ALU = mybir.AluOpType
TDT = mybir.dt.bfloat16  # transport/table dtype


# ----------------------------------------------------------------- host prep


def _ell_build(src_g, dst_core, dst_loc, self_g, n_cores, nloc, padrow):
    """Shared-K ELL: returns (K per tile, per-core int32 [P, sum(K)] arrays,
    p-major-global: element [p, koff[t]+k] = slot k of local row t*128+p)."""
    nt = nloc // P
    counts = np.zeros((n_cores, nloc), np.int64)
    np.add.at(counts, (dst_core, dst_loc), 1)
    n_self = 0 if self_g is None else 1
    cmax = counts.reshape(n_cores, nt, P).max(axis=(0, 2))
    K = (cmax + n_self).astype(np.int64)
    order = np.lexsort((dst_loc, dst_core))
    sc, sl, sg = dst_core[order], dst_loc[order], src_g[order]
    key = sc.astype(np.int64) * nloc + sl
    is_start = np.r_[True, key[1:] != key[:-1]] if len(key) else np.array([], bool)
    run_starts = np.flatnonzero(is_start)
    run_len = np.diff(np.r_[run_starts, len(key)])
    pos_in_run = np.arange(len(key)) - np.repeat(run_starts, run_len)
    koff = np.r_[0, np.cumsum(K)]
    sk = int(koff[-1])
    idx_arrs = []
    for c in range(n_cores):
        arr = np.full((P, sk), padrow[c], np.int64)
        m = sc == c
        loc, pos, gidx = sl[m], pos_in_run[m], sg[m]
        t = loc // P
        p = loc % P
        arr[p, koff[t] + pos + n_self] = gidx
        if n_self:
            allt = np.arange(nloc) // P
            allp = np.arange(nloc) % P
            arr[allp, koff[allt]] = self_g[c]
        idx_arrs.append(arr.astype(np.int32))
    return K.tolist(), idx_arrs


def _plane(vals_loc, nt):
    """[nloc] local-row vector -> [P, nt] plane (local row t*128+p -> [p, t])."""
    return np.ascontiguousarray(vals_loc.reshape(nt, P).T)


def prep(inputs, n_cores=8):
    x = np.asarray(inputs["x"], np.float32)
    ei = np.asarray(inputs["edge_index"], np.int64)
    eih = np.asarray(inputs["edge_index_hop"], np.int64)
    perm = np.asarray(inputs["perm"], np.int64)
    W1 = np.asarray(inputs["W1"], np.float32)
    W2 = np.asarray(inputs["W2"], np.float32)
    W3 = np.asarray(inputs["W3"], np.float32)
    M1 = np.asarray(inputs["M1"], np.float32)
    M2 = np.asarray(inputs["M2"], np.float32)
    Wc = np.asarray(inputs["Wc"], np.float32)
    Wd0 = np.asarray(inputs["Wd"], np.float32)[0]
    for bname in ("b1", "b2", "b3", "mb1", "mb2", "bc"):
        assert np.abs(np.asarray(inputs[bname])).max() == 0.0, (
            f"nonzero bias {bname} not supported by this kernel build"
        )

    N, n_feat = x.shape
    D = W1.shape[1]
    ncls = Wc.shape[1]
    max_real = -(-N // n_cores)
    nloc = -(-(max_real + 1) // P) * P
    nt = nloc // P
    ng = n_cores * nloc

    deg = np.bincount(ei[1], minlength=N).astype(np.float32) + 1.0
    degh = np.bincount(eih[1], minlength=N).astype(np.float32) + 1.0
    dis = 1.0 / np.sqrt(deg)
    dish = 1.0 / np.sqrt(degh)

    order = np.argsort(-deg, kind="stable")
    core_of = np.empty(N, np.int64)
    loc_of = np.empty(N, np.int64)
    core_of[order] = np.arange(N) % n_cores
    loc_of[order] = np.arange(N) // n_cores
    gl = core_of * nloc + loc_of
    padrow = [c * nloc + nloc - 1 for c in range(n_cores)]

    nat = np.full((n_cores, nloc), -1, np.int64)
    nat[core_of, loc_of] = np.arange(N)

    # hop order: per-core resort by hop degree desc (pads last)
    hkey = np.where(nat >= 0, -degh[np.maximum(nat, 0)], 1.0)
    hord = np.argsort(hkey, axis=1, kind="stable")
    hpos = np.argsort(hord, axis=1)

    selfg_pi = np.where(
        nat >= 0,
        np.arange(n_cores)[:, None] * nloc + np.arange(nloc)[None, :],
        np.array(padrow)[:, None],
    )
    # graph-g passes add the self term via a direct DMA of the local shard
    # (saves one ELL slot per tile on 3 passes); hop pass keeps self in ELL
    # (hop order makes the local slice non-contiguous).
    K1, idx1 = _ell_build(
        gl[ei[0]], core_of[ei[1]], loc_of[ei[1]], None, n_cores, nloc, padrow
    )
    selfg_h = np.take_along_axis(selfg_pi, hord, axis=1)
    K3, idx3 = _ell_build(
        gl[eih[0]],
        core_of[eih[1]],
        hpos[core_of[eih[1]], loc_of[eih[1]]],
        selfg_h,
        n_cores,
        nloc,
        padrow,
    )

    in_maps = []
    for c in range(n_cores):
        natc = nat[c]
        real = natc >= 0
        xs = np.zeros((nloc, n_feat), np.float32)
        xs[real] = x[natc[real]]
        dis_c = np.ones(nloc, np.float32)
        dis_c[real] = dis[natc[real]]
        dish_pi = np.ones(nloc, np.float32)
        dish_pi[real] = dish[natc[real]]
        dishh = np.ones(nloc, np.float32)
        hnat = natc[hord[c]]
        hreal = hnat >= 0
        dishh[hreal] = dish[hnat[hreal]]
        gperm = np.full(nloc, padrow[c], np.int64)
        ratio = np.ones(nloc, np.float32)
        pv = perm[natc[real]]
        gperm[real] = gl[pv]
        ratio[real] = dis[natc[real]] / dis[pv]
        in_maps.append(
            {
                "xT": np.ascontiguousarray(xs.T).astype(ml_dtypes.bfloat16),
                "dis_p": _plane(dis_c, nt),
                "dis2_p": _plane(dis_c * dis_c, nt),
                "dishp_p": _plane(dish_pi, nt),
                "dishh_p": _plane(dishh, nt),
                "ratio_p": _plane(ratio, nt),
                "gperm_p": _plane(gperm.astype(np.int32), nt),
                "idxR_p": _plane(hpos[c].astype(np.int32), nt),
                "idx1": idx1[c],
                "idx3": idx3[c],
                "W1": W1.astype(ml_dtypes.bfloat16),
                "W2": W2,
                "W3": W3,
                "M1": M1,
                "M2": M2,
                "Wd0": Wd0,
                "Wc": np.ascontiguousarray(Wc),
            }
        )

    meta = dict(
        n_cores=n_cores,
        nloc=nloc,
        nt=nt,
        ng=ng,
        n_feat=n_feat,
        D=D,
        ncls=ncls,
        K1=K1,
        K3=K3,
        core_of=core_of,
        loc_of=loc_of,
    )
    return in_maps, meta


# ------------------------------------------------------------- device build


def build(meta):
    n_cores = meta["n_cores"]
    nloc, nt, ng = meta["nloc"], meta["nt"], meta["ng"]
    n_feat, D, ncls = meta["n_feat"], meta["D"], meta["ncls"]
    K1, K3 = meta["K1"], meta["K3"]
    DD = 2 * D
    nfc = n_feat // P
    sk1, sk3 = sum(K1), sum(K3)
    groups = [list(range(n_cores))]

    nc = bacc.Bacc("TRN2", debug=False, num_devices=n_cores)
    shared = "Shared" if n_cores > 4 else "Local"

    xT = nc.dram_tensor("xT", [n_feat, nloc], TDT, kind="ExternalInput")
    dis_p = nc.dram_tensor("dis_p", [P, nt], F32, kind="ExternalInput")
    dis2_p = nc.dram_tensor("dis2_p", [P, nt], F32, kind="ExternalInput")
    dishp_p = nc.dram_tensor("dishp_p", [P, nt], F32, kind="ExternalInput")
    dishh_p = nc.dram_tensor("dishh_p", [P, nt], F32, kind="ExternalInput")
    ratio_p = nc.dram_tensor("ratio_p", [P, nt], F32, kind="ExternalInput")
    gperm_p = nc.dram_tensor("gperm_p", [P, nt], I32, kind="ExternalInput")
    idxR_p = nc.dram_tensor("idxR_p", [P, nt], I32, kind="ExternalInput")
    idx1 = nc.dram_tensor("idx1", [P, sk1], I32, kind="ExternalInput")
    idx3 = nc.dram_tensor("idx3", [P, sk3], I32, kind="ExternalInput")
    W1 = nc.dram_tensor("W1", [n_feat, D], TDT, kind="ExternalInput")
    W2 = nc.dram_tensor("W2", [D, D], F32, kind="ExternalInput")
    W3 = nc.dram_tensor("W3", [D, D], F32, kind="ExternalInput")
    M1 = nc.dram_tensor("M1", [D, D], F32, kind="ExternalInput")
    M2 = nc.dram_tensor("M2", [D, D], F32, kind="ExternalInput")
    Wd0 = nc.dram_tensor("Wd0", [D, D], F32, kind="ExternalInput")
    Wc = nc.dram_tensor("Wc", [D, ncls], F32, kind="ExternalInput")
    out = nc.dram_tensor("out", [nloc, ncls + 2], F32, kind="ExternalOutput")

    xw_s = nc.dram_tensor("xw_s", [nloc, D], TDT, kind="Internal")
    XW = nc.dram_tensor("XW", [ng, D], TDT, kind="Internal", addr_space=shared)
    t1_s = nc.dram_tensor("t1_s", [nloc, DD], TDT, kind="Internal")
    T1 = nc.dram_tensor("T1", [ng, DD], TDT, kind="Internal", addr_space=shared)
    zd_s = nc.dram_tensor("zd_s", [nloc, DD], TDT, kind="Internal")
    ZD = nc.dram_tensor("ZD", [ng, DD], TDT, kind="Internal", addr_space=shared)
    e1h_s = nc.dram_tensor("e1h_s", [nloc, DD], TDT, kind="Internal")
    e1d_s = nc.dram_tensor("e1d_s", [nloc, D], TDT, kind="Internal")
    E1H = nc.dram_tensor("E1H", [ng, DD], TDT, kind="Internal", addr_space=shared)
    E1D = nc.dram_tensor("E1D", [ng, D], TDT, kind="Internal", addr_space=shared)
    E2h = nc.dram_tensor("E2h", [nloc, DD], F32, kind="Internal")
    TV = nc.dram_tensor("TV", [nloc, D], F32, kind="Internal")

    with tile.TileContext(nc) as tc:
        with (
            tc.tile_pool(name="const", bufs=1) as constp,
            tc.tile_pool(name="gath", bufs=3) as gathp,
            tc.tile_pool(name="work", bufs=3) as workp,
            tc.tile_pool(name="outp", bufs=3) as outp,
            tc.tile_pool(name="psum", bufs=2, space="PSUM") as psp,
        ):
            ident = constp.tile([P, P], F32)
            make_identity(nc, ident[:])

            # resident planes + indices
            def res(t_dram, w, dt=F32, name=None):
                tl = constp.tile([P, w], dt, name=name)
                nc.sync.dma_start(tl[:], t_dram.ap())
                return tl

            disq = res(dis_p, nt, name="disq")
            dis2q = res(dis2_p, nt, name="dis2q")
            dishpq = res(dishp_p, nt, name="dishpq")
            dishhq = res(dishh_p, nt, name="dishhq")
            ratioq = res(ratio_p, nt, name="ratioq")
            gpermq = res(gperm_p, nt, I32, name="gpermq")
            idxRq = res(idxR_p, nt, I32, name="idxRq")
            idx1q = res(idx1, sk1, I32, name="idx1q")
            idx3q = res(idx3, sk3, I32, name="idx3q")

            w1t = [
                constp.tile([P, D], TDT, name=f"w1t_{i}") for i in range(nfc)
            ]
            for i in range(nfc):
                nc.sync.dma_start(w1t[i][:], W1.ap()[i * P : (i + 1) * P])
            w2t = res(W2, D, name="w2t")
            w3t = res(W3, D, name="w3t")
            m1t = res(M1, D, name="m1t")
            m2t = res(M2, D, name="m2t")
            wdt = res(Wd0, D, name="wdt")
            wct = res(Wc, ncls, name="wct")

            def rows(t):
                return slice(t * P, (t + 1) * P)

            def col(plane, t):
                return plane[:, t : t + 1]

            # ---- S0: xW1' shard
            for t in range(nt):
                ps = psp.tile([P, D], F32, tag="mm")
                for i in range(nfc):
                    xt = workp.tile([P, P], TDT, tag="xt")
                    nc.sync.dma_start(xt[:], xT.ap()[i * P : (i + 1) * P, rows(t)])
                    nc.tensor.matmul(
                        out=ps[:],
                        lhsT=xt[:],
                        rhs=w1t[i][:],
                        start=(i == 0),
                        stop=(i == nfc - 1),
                    )
                o = outp.tile([P, D], TDT, tag="s0")
                nc.vector.tensor_scalar_mul(o[:], ps[:], col(disq, t))
                nc.sync.dma_start(xw_s.ap()[rows(t)], o[:])

            nc.gpsimd.collective_compute(
                "AllGather",
                ALU.bypass,
                replica_groups=groups,
                ins=[xw_s[:].opt()],
                outs=[XW[:].opt()],
            )

            # ---- S2: T1 shard
            for t in range(nt):
                g = gathp.tile([P, D], TDT, tag="g2")
                nc.gpsimd.indirect_dma_start(
                    out=g[:],
                    out_offset=None,
                    in_=XW.ap(),
                    in_offset=bass.IndirectOffsetOnAxis(ap=col(gpermq, t), axis=0),
                )
                o = outp.tile([P, D], TDT, tag="s2")
                nc.vector.tensor_scalar_mul(o[:], g[:], col(ratioq, t))
                nc.sync.dma_start(t1_s.ap()[rows(t), D:DD], o[:])
                l = workp.tile([P, D], TDT, tag="s2l")
                nc.sync.dma_start(l[:], xw_s.ap()[rows(t)])
                nc.sync.dma_start(t1_s.ap()[rows(t), 0:D], l[:])

            nc.gpsimd.collective_compute(
                "AllGather",
                ALU.bypass,
                replica_groups=groups,
                ins=[t1_s[:].opt()],
                outs=[T1[:].opt()],
            )

            # ---- generic ELL gather: returns aggregated [P, width] tile.
            # Slots fold pairwise in bf16 (DVE 2x mode, contiguous slabs);
            # odd leftovers + optional direct-DMA'd self term combine in f32.
            def ell_tile(table, width, K, koff, idxq, self_dram=None, t=None):
                g = gathp.tile([P, max(K, 1) * width], TDT, tag="ge")
                for k in range(K):
                    nc.gpsimd.indirect_dma_start(
                        out=g[:, k * width : (k + 1) * width],
                        out_offset=None,
                        in_=table.ap(),
                        in_offset=bass.IndirectOffsetOnAxis(
                            ap=idxq[:, koff + k : koff + k + 1], axis=0
                        ),
                    )
                lt = None
                if self_dram is not None:
                    lt = workp.tile([P, width], TDT, tag="selfl")
                    nc.sync.dma_start(lt[:], self_dram.ap()[rows(t)])
                k = K
                extras = []
                while k > 1:
                    h = k // 2
                    nc.vector.tensor_tensor(
                        out=g[:, 0 : h * width],
                        in0=g[:, 0 : h * width],
                        in1=g[:, h * width : 2 * h * width],
                        op=ALU.add,
                    )
                    if k & 1:
                        extras.append(2 * h)
                    k = h
                terms = []
                if K >= 1:
                    terms.append(g[:, 0:width])
                terms += [g[:, e * width : (e + 1) * width] for e in extras]
                if lt is not None:
                    terms.append(lt[:])
                s = workp.tile([P, width], F32, tag="se")
                if len(terms) == 1:
                    nc.vector.tensor_copy(s[:], terms[0])
                else:
                    nc.vector.tensor_tensor(
                        out=s[:], in0=terms[0], in1=terms[1], op=ALU.add
                    )
                    for x in terms[2:]:
                        nc.vector.tensor_tensor(out=s[:], in0=s[:], in1=x, op=ALU.add)
                return s

            # ---- G1: zd = relu(dis2 * sum) -> zd_s
            koff = 0
            for t in range(nt):
                s = ell_tile(T1, DD, K1[t], koff, idx1q, self_dram=t1_s, t=t)
                koff += K1[t]
                o = outp.tile([P, DD], TDT, tag="ze")
                nc.vector.tensor_scalar(
                    o[:], s[:], col(dis2q, t), 0.0, ALU.mult, ALU.max
                )
                nc.sync.dma_start(zd_s.ap()[rows(t)], o[:])

            nc.gpsimd.collective_compute(
                "AllGather",
                ALU.bypass,
                replica_groups=groups,
                ins=[zd_s[:].opt()],
                outs=[ZD[:].opt()],
            )

            # ---- G2: S @ W2, three shipped variants
            koff = 0
            for t in range(nt):
                s = ell_tile(ZD, DD, K1[t], koff, idx1q, self_dram=zd_s, t=t)
                koff += K1[t]
                e1h = outp.tile([P, DD], TDT, tag="e1h")
                e1d = outp.tile([P, D], TDT, tag="e1d")
                e1p = outp.tile([P, D], F32, tag="e1p")
                for h in range(2):
                    tp = psp.tile([P, P], F32, tag="t", bufs=3)
                    nc.tensor.transpose(
                        out=tp[:], in_=s[:, h * D : (h + 1) * D], identity=ident[:]
                    )
                    tps = workp.tile([P, P], F32, tag="tps")
                    nc.vector.tensor_copy(tps[:], tp[:])
                    mm = psp.tile([P, D], F32, tag="m", bufs=3)
                    nc.tensor.matmul(
                        out=mm[:], lhsT=tps[:], rhs=w2t[:], start=True, stop=True
                    )
                    # e1 = relu(dis * mm)
                    eh = workp.tile([P, D], F32, tag="eh")
                    nc.vector.tensor_scalar(
                        eh[:], mm[:], col(disq, t), 0.0, ALU.mult, ALU.max
                    )
                    nc.vector.tensor_scalar_mul(
                        e1h[:, h * D : (h + 1) * D], eh[:], col(dishpq, t)
                    )
                    if h == 0:
                        nc.vector.tensor_copy(e1p[:], eh[:])
                        nc.vector.tensor_scalar_mul(e1d[:], eh[:], col(disq, t))
                nc.sync.dma_start(e1h_s.ap()[rows(t)], e1h[:])
                nc.sync.dma_start(e1d_s.ap()[rows(t)], e1d[:])
                # S12 folded in: MLP + tvec on the in-SBUF e1p tile; its
                # Tensor/Vector ops hide under this loop's gather issue.
                tpa = psp.tile([P, P], F32, tag="t", bufs=3)
                nc.tensor.transpose(out=tpa[:], in_=e1p[:], identity=ident[:])
                tpsa = workp.tile([P, P], F32, tag="tps")
                nc.vector.tensor_copy(tpsa[:], tpa[:])
                mma = psp.tile([P, D], F32, tag="m", bufs=3)
                nc.tensor.matmul(out=mma[:], lhsT=tpsa[:], rhs=m1t[:], start=True, stop=True)
                u = workp.tile([P, D], F32, tag="ml_u")
                nc.scalar.activation(u[:], mma[:], AF.Relu)
                tpb = psp.tile([P, P], F32, tag="t", bufs=3)
                nc.tensor.transpose(out=tpb[:], in_=u[:], identity=ident[:])
                tpsb = workp.tile([P, P], F32, tag="tps")
                nc.vector.tensor_copy(tpsb[:], tpb[:])
                mmb = psp.tile([P, D], F32, tag="m", bufs=3)
                nc.tensor.matmul(out=mmb[:], lhsT=tpsb[:], rhs=m2t[:], start=True, stop=True)
                e3 = workp.tile([P, D], F32, tag="ml_e3")
                nc.vector.tensor_copy(e3[:], mmb[:])
                tpc = psp.tile([P, P], F32, tag="t", bufs=3)
                nc.tensor.transpose(out=tpc[:], in_=e3[:], identity=ident[:])
                tpsc = workp.tile([P, P], F32, tag="tps")
                nc.vector.tensor_copy(tpsc[:], tpc[:])
                mmc = psp.tile([P, D], F32, tag="m", bufs=3)
                nc.tensor.matmul(out=mmc[:], lhsT=tpsc[:], rhs=wdt[:], start=True, stop=True)
                tv = outp.tile([P, D], F32, tag="ml_tv")
                nc.vector.tensor_copy(tv[:], mmc[:])
                nc.sync.dma_start(TV.ap()[rows(t)], tv[:])

            # E1D ships first: G4's gathers depend only on it, and they hide
            # the bigger E1H AllGather that G3 needs.
            nc.gpsimd.collective_compute(
                "AllGather",
                ALU.bypass,
                replica_groups=groups,
                ins=[e1d_s[:].opt()],
                outs=[E1D[:].opt()],
            )
            nc.gpsimd.collective_compute(
                "AllGather",
                ALU.bypass,
                replica_groups=groups,
                ins=[e1h_s[:].opt()],
                outs=[E1H[:].opt()],
            )

            # ---- G4: cls = (dis * sum) @ Wc -> out[:, :ncls]
            koff = 0
            for t in range(nt):
                s = ell_tile(E1D, D, K1[t], koff, idx1q, self_dram=e1d_s, t=t)
                koff += K1[t]
                sc_ = workp.tile([P, D], F32, tag="c_s")
                nc.vector.tensor_scalar_mul(sc_[:], s[:], col(disq, t))
                tp = psp.tile([P, P], F32, tag="t", bufs=3)
                nc.tensor.transpose(out=tp[:], in_=sc_[:], identity=ident[:])
                tps = workp.tile([P, P], F32, tag="tps")
                nc.vector.tensor_copy(tps[:], tp[:])
                mm = psp.tile([P, ncls], F32, tag="m", bufs=3)
                nc.tensor.matmul(out=mm[:], lhsT=tps[:], rhs=wct[:], start=True, stop=True)
                o = outp.tile([P, ncls], F32, tag="c_o")
                nc.vector.tensor_copy(o[:], mm[:])
                nc.sync.dma_start(out.ap()[rows(t), 0:ncls], o[:])

            # ---- G3: embed2{,b} = dishh * (S_h @ W3) -> E2h (hop order)
            koff = 0
            for t in range(nt):
                s = ell_tile(E1H, DD, K3[t], koff, idx3q)
                koff += K3[t]
                e2 = outp.tile([P, DD], F32, tag="e2")
                for h in range(2):
                    tp = psp.tile([P, P], F32, tag="t", bufs=3)
                    nc.tensor.transpose(
                        out=tp[:], in_=s[:, h * D : (h + 1) * D], identity=ident[:]
                    )
                    tps = workp.tile([P, P], F32, tag="tps")
                    nc.vector.tensor_copy(tps[:], tp[:])
                    mm = psp.tile([P, D], F32, tag="m", bufs=3)
                    nc.tensor.matmul(
                        out=mm[:], lhsT=tps[:], rhs=w3t[:], start=True, stop=True
                    )
                    nc.vector.tensor_scalar_mul(
                        e2[:, h * D : (h + 1) * D], mm[:], col(dishhq, t)
                    )
                nc.sync.dma_start(E2h.ap()[rows(t)], e2[:])

            # ---- S11 + S13: realign + scores
            for t in range(nt):
                e2 = gathp.tile([P, DD], F32, tag="gr")
                nc.gpsimd.indirect_dma_start(
                    out=e2[:],
                    out_offset=None,
                    in_=E2h.ap(),
                    in_offset=bass.IndirectOffsetOnAxis(ap=col(idxRq, t), axis=0),
                )
                tv = workp.tile([P, D], F32, tag="sc_tv")
                nc.sync.dma_start(tv[:], TV.ap()[rows(t)])
                pr = workp.tile([P, DD], F32, tag="sc_pr")
                nc.vector.tensor_mul(pr[:, 0:D], tv[:], e2[:, 0:D])
                nc.vector.tensor_mul(pr[:, D:DD], tv[:], e2[:, D:DD])
                rs = workp.tile([P, 2], F32, tag="sc_rs")
                nc.vector.tensor_reduce(
                    out=rs[:],
                    in_=pr[:].rearrange("p (h d) -> p h d", h=2),
                    axis=mybir.AxisListType.X,
                    op=ALU.add,
                )
                sg = outp.tile([P, 2], F32, tag="sc_sg")
                nc.scalar.activation(sg[:], rs[:], AF.Sigmoid)
                nc.sync.dma_start(out.ap()[rows(t), ncls : ncls + 2], sg[:])

    nc.compile()
    return nc


def assemble(results, meta):
    n_cores = meta["n_cores"]
    N = len(meta["core_of"])
    ncls = meta["ncls"]
    out = np.empty((N, ncls + 2), np.float32)
    for c in range(n_cores):
        oc = results[c]["out"]
        m = meta["core_of"] == c
        out[m] = oc[meta["loc_of"][m]]
    return out


# ------------------------------------------------------------------ entry


_CACHE = {}


def kernel(**inputs):
    """Full-input entry point: shards across 8 NeuronCores internally.

    Expects the nn_MixModel input dict (x, edge_index, edge_index_hop, y,
    perm, W1..Wd); returns the full [N, n_cls+2] float32 output.
    """
    n_cores = 8
    in_maps, meta = prep(inputs, n_cores)
    key = (meta["nloc"], tuple(meta["K1"]), tuple(meta["K3"]))
    nc = _CACHE.get(key)
    if nc is None:
        nc = build(meta)
        _CACHE[key] = nc
    res = bass_utils.run_bass_kernel_spmd(
        nc, in_maps, core_ids=list(range(n_cores)), trace=False
    )
    return assemble(res.results, meta)



# revision 19
# speedup vs baseline: 1.1448x; 1.0766x over previous
"""8-core Trainium2 Bass kernel for nn_MixModel (GCN mix model) — v4.

Aggregation = flat dma_gather (1024-row chunks, int16 window slices) +
segment-matrix matmuls accumulated in PSUM:

 - Edges (minus self loops) are grouped by (supertile of 8 dst tiles, src
   window of ng/4 rows, dst tile); each (tile, window) run is padded to a
   multiple of 128 so every 128-edge block belongs to exactly one dst tile
   (keeps the SPMD program identical across cores).
 - dma_gather pulls the blocks' table rows (bf16, 512B/256B) at ~8ns/row of
   GpSimd issue time (vs 11ns/row for 128-row indirect DMAs).
 - Per block, one matmul with a host-built 0/1 segment matrix (lhsT, bf16)
   accumulates rows into the dst tile's PSUM bank: out[dst_row, feat] +=
   sum_p seg[p, dst_row] * g[p, feat]. 8 PSUM tiles live per supertile.
 - Self terms ride a direct DMA of the local shard + one f32 add at drain.
 - No shared-K ELL, no hop reordering: the hop pass (G3) runs in pi order,
   so scores fuse into its drain and v2's S11 realign pass is gone.

Stages (per core):
  S0   xW1' shard = (x_sh @ W1) * dis_sh (bf16) -> AG0 -> XW [NG,128]
  S2   T1 shard = [xW1'_loc | gather(XW, gperm)*ratio] ; AG1 -> T1 [NG,256]
  G1   blockmm pass over graph-g on T1 -> zd = relu(dis2 * s) -> AG2 -> ZD
  G2   blockmm pass over graph-g on ZD -> e1{,b} = relu(dis * (s_h @ W2));
       ship [e1*dishp|e1b*dishp] -> E1H ; e1*dis -> E1D ; MLP+tvec fused
  G4   blockmm pass over graph-g on E1D -> cls = (dis*s)@Wc -> out[:, :10]
  G3   blockmm pass over graph-h on E1H -> e2 = dishp * (s_h @ W3);
       scores = sigmoid(rowsum(tvec * e2{,b})) -> out[:, 10:12]
"""

import ml_dtypes
import numpy as np

import concourse.bacc as bacc
import concourse.bass as bass
import concourse.mybir as mybir
import concourse.tile as tile
from concourse import bass_utils
from concourse.masks import make_identity

P = 128
F32 = mybir.dt.float32
I16 = mybir.dt.int16
I32 = mybir.dt.int32
AF = mybir.ActivationFunctionType
ALU = mybir.AluOpType
TDT = mybir.dt.bfloat16  # transport/table dtype

N_WIN = 4
CH = 1024   # dma_gather cap (8 Q7 cores x 128 in-flight descriptors)
ST = 4      # dst tiles per supertile (PSUM banks are 2KB units)


# ----------------------------------------------------------------- host prep


def _wrap16(idx):
    """[n] -> [128, n//16] int16: value i at [i%16, i//16], replicated x8."""
    n = len(idx)
    w = np.ascontiguousarray(idx.reshape(n // 16, 16).T).astype(np.int16)
    return np.tile(w, (8, 1))


def _plane(vals_loc, nt):
    return np.ascontiguousarray(vals_loc.reshape(nt, P).T)


def _blk_build(per_core_edges, n_cores, nloc, ng):
    """Block structure + per-core gather-idx / segmat arrays.

    per_core_edges: [(src_gl, dst_loc)] WITHOUT self loops.
    Returns (sts, gidx_arrs, seg_arrs, total_rows, nblk):
      sts: list of {tiles: [(t, has_blocks)], chunks: [(w, nrows,
           [(tile, start, stop)])]}
      gidx_arrs: per-core [128, total//16] int16 (window-relative)
      seg_arrs: per-core [128, nblk*128] bf16 segment matrices
    """
    wrow = ng // N_WIN
    nt = nloc // P

    counts = np.zeros((n_cores, nt, N_WIN), np.int64)
    grouped = []
    for c, (src, dst) in enumerate(per_core_edges):
        w = src // wrow
        t = dst // P
        np.add.at(counts[c], (t, w), 1)
        o = np.lexsort((w, t))
        grouped.append((t[o], w[o], src[o], dst[o]))
    S = -(-counts.max(axis=0) // P) * P  # [nt, N_WIN] common padded sizes

    nst = -(-nt // ST)
    sts = []
    total = 0
    nblk = 0
    for si in range(nst):
        tiles = list(range(si * ST, min((si + 1) * ST, nt)))
        first_left = {t: True for t in tiles}
        # which block is the last for each tile: count blocks per tile
        blocks_of = {t: int(sum(S[t])) // P for t in tiles}
        seen = {t: 0 for t in tiles}
        chunks = []
        for w in range(N_WIN):
            run = [(t, int(S[t, w])) for t in tiles if S[t, w] > 0]
            # split into chunks of <= CH rows (128-aligned)
            pend = []
            pend_rows = 0
            for t, sz in run:
                while sz > 0:
                    take = min(sz, CH - pend_rows)
                    nb = take // P
                    for _ in range(nb):
                        st_f = first_left[t]
                        first_left[t] = False
                        seen[t] += 1
                        sp_f = seen[t] == blocks_of[t]
                        pend.append((t, st_f, sp_f))
                    pend_rows += take
                    sz -= take
                    if pend_rows == CH:
                        chunks.append((w, pend_rows, pend))
                        pend, pend_rows = [], 0
            if pend_rows:
                chunks.append((w, pend_rows, pend))
        for w, nrows, blks in chunks:
            total += nrows
            nblk += len(blks)
        sts.append(
            {
                "tiles": [(t, blocks_of[t] > 0) for t in tiles],
                "chunks": chunks,
            }
        )

    # per-core fills
    gidx_arrs, seg_arrs = [], []
    for c in range(n_cores):
        tg, wg, sg, dg = grouped[c]
        key = tg * N_WIN + wg
        # start offset of each (t, w) group in the sorted edge arrays
        starts = np.searchsorted(key, np.arange(nt * N_WIN) )
        ends = np.searchsorted(key, np.arange(nt * N_WIN) + 1)
        gflat = np.zeros(total, np.int64)
        segp = np.zeros(total, np.int64)   # partition index per row (pos%128)
        segc = np.full(total, -1, np.int64)  # dst col per row (-1 = pad)
        pos = 0
        for stv in sts:
            cursor = {}
            for w, nrows, blks in stv["chunks"]:
                for bj, (t, _sf, _pf) in enumerate(blks):
                    kk = t * N_WIN + w
                    cur = cursor.get(kk, starts[kk])
                    n = min(ends[kk] - cur, P)
                    base = pos + bj * P
                    if n > 0:
                        gflat[base : base + n] = sg[cur : cur + n] - w * wrow
                        segc[base : base + n] = dg[cur : cur + n] - t * P
                    cursor[kk] = cur + n
                    pos += 0
                pos += nrows
        assert pos == total
        blkid = np.arange(total) // P
        prow = np.arange(total) % P
        seg = np.zeros((P, nblk * P), ml_dtypes.bfloat16)
        m = segc >= 0
        seg[prow[m], blkid[m] * P + segc[m]] = 1.0
        gidx_arrs.append(_wrap16(gflat))
        seg_arrs.append(seg)

    return sts, gidx_arrs, seg_arrs, total, nblk


def prep(inputs, n_cores=8):
    x = np.asarray(inputs["x"], np.float32)
    ei = np.asarray(inputs["edge_index"], np.int64)
    eih = np.asarray(inputs["edge_index_hop"], np.int64)
    perm = np.asarray(inputs["perm"], np.int64)
    W1 = np.asarray(inputs["W1"], np.float32)
    W2 = np.asarray(inputs["W2"], np.float32)
    W3 = np.asarray(inputs["W3"], np.float32)
    M1 = np.asarray(inputs["M1"], np.float32)
    M2 = np.asarray(inputs["M2"], np.float32)
    Wc = np.asarray(inputs["Wc"], np.float32)
    Wd0 = np.asarray(inputs["Wd"], np.float32)[0]
    for bname in ("b1", "b2", "b3", "mb1", "mb2", "bc"):
        assert np.abs(np.asarray(inputs[bname])).max() == 0.0, (
            f"nonzero bias {bname} not supported by this kernel build"
        )

    N, n_feat = x.shape
    D = W1.shape[1]
    ncls = Wc.shape[1]
    max_real = -(-N // n_cores)
    nloc = -(-(max_real + 1) // P) * P
    nt = nloc // P
    ng = n_cores * nloc
    assert ng % N_WIN == 0 and ng // N_WIN < 2**15

    deg = np.bincount(ei[1], minlength=N).astype(np.float32) + 1.0
    degh = np.bincount(eih[1], minlength=N).astype(np.float32) + 1.0
    dis = 1.0 / np.sqrt(deg)
    dish = 1.0 / np.sqrt(degh)

    order = np.argsort(-deg, kind="stable")
    core_of = np.empty(N, np.int64)
    loc_of = np.empty(N, np.int64)
    core_of[order] = np.arange(N) % n_cores
    loc_of[order] = np.arange(N) // n_cores
    gl = core_of * nloc + loc_of
    padrow = [c * nloc + nloc - 1 for c in range(n_cores)]

    nat = np.full((n_cores, nloc), -1, np.int64)
    nat[core_of, loc_of] = np.arange(N)

    def edge_lists(e2n):
        src, dst = e2n
        return [
            (gl[src[core_of[dst] == c]], loc_of[dst[core_of[dst] == c]])
            for c in range(n_cores)
        ]

    sts_g, gidx_g, seg_g, tot_g, nblk_g = _blk_build(
        edge_lists(ei), n_cores, nloc, ng
    )
    sts_h, gidx_h, seg_h, tot_h, nblk_h = _blk_build(
        edge_lists(eih), n_cores, nloc, ng
    )

    in_maps = []
    for c in range(n_cores):
        natc = nat[c]
        real = natc >= 0
        xs = np.zeros((nloc, n_feat), np.float32)
        xs[real] = x[natc[real]]
        dis_c = np.ones(nloc, np.float32)
        dis_c[real] = dis[natc[real]]
        dishp = np.ones(nloc, np.float32)
        dishp[real] = dish[natc[real]]
        gperm = np.full(nloc, padrow[c], np.int64)
        ratio = np.ones(nloc, np.float32)
        pv = perm[natc[real]]
        gperm[real] = gl[pv]
        ratio[real] = dis[natc[real]] / dis[pv]
        in_maps.append(
            {
                "xT": np.ascontiguousarray(xs.T).astype(ml_dtypes.bfloat16),
                "dis_p": _plane(dis_c, nt),
                "dis2_p": _plane(dis_c * dis_c, nt),
                "dishp_p": _plane(dishp, nt),
                "ratio_p": _plane(ratio, nt),
                "gperm_p": _plane(gperm.astype(np.int32), nt),
                "gidx_g": gidx_g[c],
                "seg_g": seg_g[c],
                "gidx_h": gidx_h[c],
                "seg_h": seg_h[c],
                "W1": W1.astype(ml_dtypes.bfloat16),
                "W2": W2,
                "W3": W3,
                "M1": M1,
                "M2": M2,
                "Wd0": Wd0,
                "Wc": np.ascontiguousarray(Wc),
            }
        )

    meta = dict(
        n_cores=n_cores,
        nloc=nloc,
        nt=nt,
        ng=ng,
        n_feat=n_feat,
        D=D,
        ncls=ncls,
        sts_g=sts_g,
        sts_h=sts_h,
        tot_g=tot_g,
        tot_h=tot_h,
        nblk_g=nblk_g,
        nblk_h=nblk_h,
        core_of=core_of,
        loc_of=loc_of,
    )
    return in_maps, meta


# ------------------------------------------------------------- device build


def build(meta):
    n_cores = meta["n_cores"]
    nloc, nt, ng = meta["nloc"], meta["nt"], meta["ng"]
    n_feat, D, ncls = meta["n_feat"], meta["D"], meta["ncls"]
    DD = 2 * D
    nfc = n_feat // P
    wrow = ng // N_WIN
    groups = [list(range(n_cores))]

    nc = bacc.Bacc("TRN2", debug=False, num_devices=n_cores)
    shared = "Shared" if n_cores > 4 else "Local"

    xT = nc.dram_tensor("xT", [n_feat, nloc], TDT, kind="ExternalInput")
    dis_p = nc.dram_tensor("dis_p", [P, nt], F32, kind="ExternalInput")
    dis2_p = nc.dram_tensor("dis2_p", [P, nt], F32, kind="ExternalInput")
    dishp_p = nc.dram_tensor("dishp_p", [P, nt], F32, kind="ExternalInput")
    ratio_p = nc.dram_tensor("ratio_p", [P, nt], F32, kind="ExternalInput")
    gperm_p = nc.dram_tensor("gperm_p", [P, nt], I32, kind="ExternalInput")
    gidx_g = nc.dram_tensor("gidx_g", [P, meta["tot_g"] // 16], I16, kind="ExternalInput")
    seg_g = nc.dram_tensor("seg_g", [P, meta["nblk_g"] * P], TDT, kind="ExternalInput")
    gidx_h = nc.dram_tensor("gidx_h", [P, meta["tot_h"] // 16], I16, kind="ExternalInput")
    seg_h = nc.dram_tensor("seg_h", [P, meta["nblk_h"] * P], TDT, kind="ExternalInput")
    W1 = nc.dram_tensor("W1", [n_feat, D], TDT, kind="ExternalInput")
    W2 = nc.dram_tensor("W2", [D, D], F32, kind="ExternalInput")
    W3 = nc.dram_tensor("W3", [D, D], F32, kind="ExternalInput")
    M1 = nc.dram_tensor("M1", [D, D], F32, kind="ExternalInput")
    M2 = nc.dram_tensor("M2", [D, D], F32, kind="ExternalInput")
    Wd0 = nc.dram_tensor("Wd0", [D, D], F32, kind="ExternalInput")
    Wc = nc.dram_tensor("Wc", [D, ncls], F32, kind="ExternalInput")
    out = nc.dram_tensor("out", [nloc, ncls + 2], F32, kind="ExternalOutput")

    xw_s = nc.dram_tensor("xw_s", [nloc, D], TDT, kind="Internal")
    XW = nc.dram_tensor("XW", [ng, D], TDT, kind="Internal", addr_space=shared)
    t1_s = nc.dram_tensor("t1_s", [nloc, DD], TDT, kind="Internal")
    T1 = nc.dram_tensor("T1", [ng, DD], TDT, kind="Internal", addr_space=shared)
    zd_s = nc.dram_tensor("zd_s", [nloc, DD], TDT, kind="Internal")
    ZD = nc.dram_tensor("ZD", [ng, DD], TDT, kind="Internal", addr_space=shared)
    e1h_s = nc.dram_tensor("e1h_s", [nloc, DD], TDT, kind="Internal")
    e1d_s = nc.dram_tensor("e1d_s", [nloc, D], TDT, kind="Internal")
    E1H = nc.dram_tensor("E1H", [ng, DD], TDT, kind="Internal", addr_space=shared)
    E1D = nc.dram_tensor("E1D", [ng, D], TDT, kind="Internal", addr_space=shared)
    TV = nc.dram_tensor("TV", [nloc, D], F32, kind="Internal")

    with tile.TileContext(nc) as tc:
        with (
            tc.tile_pool(name="const", bufs=1) as constp,
            tc.tile_pool(name="gath", bufs=4) as gathp,
            tc.tile_pool(name="segp", bufs=4) as segp,
            tc.tile_pool(name="idx", bufs=4) as idxp,
            tc.tile_pool(name="work", bufs=3) as workp,
            tc.tile_pool(name="outp", bufs=3) as outp,
            tc.tile_pool(name="psum", bufs=2, space="PSUM") as psp,
        ):
            ident = constp.tile([P, P], F32)
            make_identity(nc, ident[:])

            def res(t_dram, w, dt=F32, name=None):
                tl = constp.tile([P, w], dt, name=name)
                nc.sync.dma_start(tl[:], t_dram.ap())
                return tl

            disq = res(dis_p, nt, name="disq")
            dis2q = res(dis2_p, nt, name="dis2q")
            dishpq = res(dishp_p, nt, name="dishpq")
            ratioq = res(ratio_p, nt, name="ratioq")
            gpermq = res(gperm_p, nt, I32, name="gpermq")

            w1t = [constp.tile([P, D], TDT, name=f"w1t_{i}") for i in range(nfc)]
            for i in range(nfc):
                nc.sync.dma_start(w1t[i][:], W1.ap()[i * P : (i + 1) * P])
            w2t = res(W2, D, name="w2t")
            w3t = res(W3, D, name="w3t")
            m1t = res(M1, D, name="m1t")
            m2t = res(M2, D, name="m2t")
            wdt = res(Wd0, D, name="wdt")
            wct = res(Wc, ncls, name="wct")

            def rows(t):
                return slice(t * P, (t + 1) * P)

            def col(plane, t):
                return plane[:, t : t + 1]

            # ---- blockmm pass: gather chunks + segmat matmuls into PSUM
            def run_pass(table, W, sts, gidx_t, seg_t, self_dram, compute_fn):
                colpos = 0
                blkpos = 0
                for stv in sts:
                    pts = {}
                    for t, has in stv["tiles"]:
                        if has:
                            pt = psp.tile(
                                [P, DD], F32, tag=f"pt{t % ST}", bufs=1,
                                name=f"pt{t % ST}",
                            )
                            pts[t] = pt
                    for w, nrows, blks in stv["chunks"]:
                        gi = idxp.tile([P, CH // 16], I16, tag="gi")
                        nc.sync.dma_start(
                            gi[:, : nrows // 16],
                            gidx_t.ap()[:, colpos : colpos + nrows // 16],
                        )
                        slab = segp.tile([P, (CH // P) * P], TDT, tag="slab")
                        nc.sync.dma_start(
                            slab[:, : len(blks) * P],
                            seg_t.ap()[:, blkpos * P : (blkpos + len(blks)) * P],
                        )
                        g = gathp.tile([P, (CH // P) * DD], TDT, tag="g")
                        gv = g[:, : (nrows // P) * W].rearrange(
                            "p (b w) -> p b w", w=W
                        )
                        nc.gpsimd.dma_gather(
                            out_ap=gv,
                            in_ap=table.ap()[w * wrow : (w + 1) * wrow],
                            idxs_ap=gi[:, : nrows // 16],
                            num_idxs=nrows,
                            num_idxs_reg=nrows,
                            elem_size=W,
                        )
                        for j, (t, st_f, sp_f) in enumerate(blks):
                            nc.tensor.matmul(
                                out=pts[t][:, :W],
                                lhsT=slab[:, j * P : (j + 1) * P],
                                rhs=gv[:, j, :],
                                start=st_f,
                                stop=sp_f,
                            )
                        colpos += nrows // 16
                        blkpos += len(blks)
                    for t, has in stv["tiles"]:
                        lt = workp.tile([P, W], TDT, tag="selfl")
                        nc.sync.dma_start(lt[:], self_dram.ap()[rows(t)])
                        s = workp.tile([P, W], F32, tag="se")
                        if has:
                            nc.vector.tensor_tensor(
                                out=s[:], in0=pts[t][:, :W], in1=lt[:], op=ALU.add
                            )
                        else:
                            nc.vector.tensor_copy(s[:], lt[:])
                        compute_fn(t, s)

            # ---- S0: xW1' shard
            for t in range(nt):
                ps = psp.tile([P, D], F32, tag="m", bufs=2)
                for i in range(nfc):
                    xt = workp.tile([P, P], TDT, tag="xt")
                    nc.sync.dma_start(xt[:], xT.ap()[i * P : (i + 1) * P, rows(t)])
                    nc.tensor.matmul(
                        out=ps[:],
                        lhsT=xt[:],
                        rhs=w1t[i][:],
                        start=(i == 0),
                        stop=(i == nfc - 1),
                    )
                o = outp.tile([P, D], TDT, tag="s0")
                nc.vector.tensor_scalar_mul(o[:], ps[:], col(disq, t))
                nc.sync.dma_start(xw_s.ap()[rows(t)], o[:])

            nc.gpsimd.collective_compute(
                "AllGather",
                ALU.bypass,
                replica_groups=groups,
                ins=[xw_s[:].opt()],
                outs=[XW[:].opt()],
            )

            # ---- S2: T1 shard (perm gather via indirect DMA, deg=1)
            for t in range(nt):
                gp_ = gathp.tile([P, D], TDT, tag="g2")
                nc.gpsimd.indirect_dma_start(
                    out=gp_[:],
                    out_offset=None,
                    in_=XW.ap(),
                    in_offset=bass.IndirectOffsetOnAxis(ap=col(gpermq, t), axis=0),
                )
                o = outp.tile([P, D], TDT, tag="s2")
                nc.vector.tensor_scalar_mul(o[:], gp_[:], col(ratioq, t))
                nc.sync.dma_start(t1_s.ap()[rows(t), D:DD], o[:])
                l = workp.tile([P, D], TDT, tag="s2l")
                nc.sync.dma_start(l[:], xw_s.ap()[rows(t)])
                nc.sync.dma_start(t1_s.ap()[rows(t), 0:D], l[:])

            nc.gpsimd.collective_compute(
                "AllGather",
                ALU.bypass,
                replica_groups=groups,
                ins=[t1_s[:].opt()],
                outs=[T1[:].opt()],
            )

            # ---- G1
            def g1_compute(t, s):
                o = outp.tile([P, DD], TDT, tag="ze")
                nc.vector.tensor_scalar(
                    o[:], s[:], col(dis2q, t), 0.0, ALU.mult, ALU.max
                )
                nc.sync.dma_start(zd_s.ap()[rows(t)], o[:])

            run_pass(T1, DD, meta["sts_g"], gidx_g, seg_g, t1_s, g1_compute)

            nc.gpsimd.collective_compute(
                "AllGather",
                ALU.bypass,
                replica_groups=groups,
                ins=[zd_s[:].opt()],
                outs=[ZD[:].opt()],
            )

            # ---- G2 (+ fused MLP/tvec)
            def g2_compute(t, s):
                e1h = outp.tile([P, DD], TDT, tag="e1h")
                e1d = outp.tile([P, D], TDT, tag="e1d")
                e1p = outp.tile([P, D], F32, tag="e1p")
                for h in range(2):
                    tp = psp.tile([P, P], F32, tag="t", bufs=2)
                    nc.tensor.transpose(
                        out=tp[:], in_=s[:, h * D : (h + 1) * D], identity=ident[:]
                    )
                    tps = workp.tile([P, P], F32, tag="tps")
                    nc.vector.tensor_copy(tps[:], tp[:])
                    mm = psp.tile([P, D], F32, tag="m", bufs=2)
                    nc.tensor.matmul(
                        out=mm[:], lhsT=tps[:], rhs=w2t[:], start=True, stop=True
                    )
                    eh = workp.tile([P, D], F32, tag="eh")
                    nc.vector.tensor_scalar(
                        eh[:], mm[:], col(disq, t), 0.0, ALU.mult, ALU.max
                    )
                    nc.vector.tensor_scalar_mul(
                        e1h[:, h * D : (h + 1) * D], eh[:], col(dishpq, t)
                    )
                    if h == 0:
                        nc.vector.tensor_copy(e1p[:], eh[:])
                        nc.vector.tensor_scalar_mul(e1d[:], eh[:], col(disq, t))
                nc.sync.dma_start(e1h_s.ap()[rows(t)], e1h[:])
                nc.sync.dma_start(e1d_s.ap()[rows(t)], e1d[:])
                tpa = psp.tile([P, P], F32, tag="t", bufs=2)
                nc.tensor.transpose(out=tpa[:], in_=e1p[:], identity=ident[:])
                tpsa = workp.tile([P, P], F32, tag="tps")
                nc.vector.tensor_copy(tpsa[:], tpa[:])
                mma = psp.tile([P, D], F32, tag="m", bufs=2)
                nc.tensor.matmul(out=mma[:], lhsT=tpsa[:], rhs=m1t[:], start=True, stop=True)
                u = workp.tile([P, D], F32, tag="ml_u")
                nc.scalar.activation(u[:], mma[:], AF.Relu)
                tpb = psp.tile([P, P], F32, tag="t", bufs=2)
                nc.tensor.transpose(out=tpb[:], in_=u[:], identity=ident[:])
                tpsb = workp.tile([P, P], F32, tag="tps")
                nc.vector.tensor_copy(tpsb[:], tpb[:])
                mmb = psp.tile([P, D], F32, tag="m", bufs=2)
                nc.tensor.matmul(out=mmb[:], lhsT=tpsb[:], rhs=m2t[:], start=True, stop=True)
                e3 = workp.tile([P, D], F32, tag="ml_e3")
                nc.vector.tensor_copy(e3[:], mmb[:])
                tpc = psp.tile([P, P], F32, tag="t", bufs=2)
                nc.tensor.transpose(out=tpc[:], in_=e3[:], identity=ident[:])
                tpsc = workp.tile([P, P], F32, tag="tps")
                nc.vector.tensor_copy(tpsc[:], tpc[:])
                mmc = psp.tile([P, D], F32, tag="m", bufs=2)
                nc.tensor.matmul(out=mmc[:], lhsT=tpsc[:], rhs=wdt[:], start=True, stop=True)
                tv = outp.tile([P, D], F32, tag="ml_tv")
                nc.vector.tensor_copy(tv[:], mmc[:])
                nc.sync.dma_start(TV.ap()[rows(t)], tv[:])

            run_pass(ZD, DD, meta["sts_g"], gidx_g, seg_g, zd_s, g2_compute)

            nc.gpsimd.collective_compute(
                "AllGather",
                ALU.bypass,
                replica_groups=groups,
                ins=[e1d_s[:].opt()],
                outs=[E1D[:].opt()],
            )
            nc.gpsimd.collective_compute(
                "AllGather",
                ALU.bypass,
                replica_groups=groups,
                ins=[e1h_s[:].opt()],
                outs=[E1H[:].opt()],
            )

            # ---- G4 (before G3: its gathers hide the E1H AllGather)
            def g4_compute(t, s):
                sc_ = workp.tile([P, D], F32, tag="c_s")
                nc.vector.tensor_scalar_mul(sc_[:], s[:], col(disq, t))
                tp = psp.tile([P, P], F32, tag="t", bufs=2)
                nc.tensor.transpose(out=tp[:], in_=sc_[:], identity=ident[:])
                tps = workp.tile([P, P], F32, tag="tps")
                nc.vector.tensor_copy(tps[:], tp[:])
                mm = psp.tile([P, ncls], F32, tag="m", bufs=2)
                nc.tensor.matmul(out=mm[:], lhsT=tps[:], rhs=wct[:], start=True, stop=True)
                o = outp.tile([P, ncls], F32, tag="c_o")
                nc.vector.tensor_copy(o[:], mm[:])
                nc.sync.dma_start(out.ap()[rows(t), 0:ncls], o[:])

            run_pass(E1D, D, meta["sts_g"], gidx_g, seg_g, e1d_s, g4_compute)

            # ---- G3 (+ fused scores)
            def g3_compute(t, s):
                e2 = workp.tile([P, DD], F32, tag="e2")
                for h in range(2):
                    tp = psp.tile([P, P], F32, tag="t", bufs=2)
                    nc.tensor.transpose(
                        out=tp[:], in_=s[:, h * D : (h + 1) * D], identity=ident[:]
                    )
                    tps = workp.tile([P, P], F32, tag="tps")
                    nc.vector.tensor_copy(tps[:], tp[:])
                    mm = psp.tile([P, D], F32, tag="m", bufs=2)
                    nc.tensor.matmul(
                        out=mm[:], lhsT=tps[:], rhs=w3t[:], start=True, stop=True
                    )
                    nc.vector.tensor_scalar_mul(
                        e2[:, h * D : (h + 1) * D], mm[:], col(dishpq, t)
                    )
                tv = workp.tile([P, D], F32, tag="sc_tv")
                nc.sync.dma_start(tv[:], TV.ap()[rows(t)])
                pr = workp.tile([P, DD], F32, tag="sc_pr")
                nc.vector.tensor_mul(pr[:, 0:D], tv[:], e2[:, 0:D])
                nc.vector.tensor_mul(pr[:, D:DD], tv[:], e2[:, D:DD])
                rs = workp.tile([P, 2], F32, tag="sc_rs")
                nc.vector.tensor_reduce(
                    out=rs[:],
                    in_=pr[:].rearrange("p (h d) -> p h d", h=2),
                    axis=mybir.AxisListType.X,
                    op=ALU.add,
                )
                sg = outp.tile([P, 2], F32, tag="sc_sg")
                nc.scalar.activation(sg[:], rs[:], AF.Sigmoid)
                nc.sync.dma_start(out.ap()[rows(t), ncls : ncls + 2], sg[:])

            run_pass(E1H, DD, meta["sts_h"], gidx_h, seg_h, e1h_s, g3_compute)

    nc.compile()
    return nc


def assemble(results, meta):
    n_cores = meta["n_cores"]
    N = len(meta["core_of"])
    ncls = meta["ncls"]
    out = np.empty((N, ncls + 2), np.float32)
    for c in range(n_cores):
        oc = results[c]["out"]
        m = meta["core_of"] == c
        out[m] = oc[meta["loc_of"][m]]
    return out


# ------------------------------------------------------------------ entry


_CACHE = {}


def kernel(**inputs):
    """Full-input entry point: shards across 8 NeuronCores internally.

    Expects the nn_MixModel input dict (x, edge_index, edge_index_hop, y,
    perm, W1..Wd); returns the full [N, n_cls+2] float32 output.
    """
    n_cores = 8
    in_maps, meta = prep(inputs, n_cores)
    key = (meta["nloc"], str(meta["sts_g"]), str(meta["sts_h"]))
    nc = _CACHE.get(key)
    if nc is None:
        nc = build(meta)
        _CACHE[key] = nc
    res = bass_utils.run_bass_kernel_spmd(
        nc, in_maps, core_ids=list(range(n_cores)), trace=False
    )
    return assemble(res.results, meta)


# revision 21
# speedup vs baseline: 1.1496x; 1.0042x over previous
"""8-core Trainium2 Bass kernel for nn_MixModel (GCN mix model) — v4.

Aggregation = flat dma_gather (1024-row chunks, int16 window slices) +
segment-matrix matmuls accumulated in PSUM:

 - Edges (minus self loops) are grouped by (supertile of 8 dst tiles, src
   window of ng/4 rows, dst tile); each (tile, window) run is padded to a
   multiple of 128 so every 128-edge block belongs to exactly one dst tile
   (keeps the SPMD program identical across cores).
 - dma_gather pulls the blocks' table rows (bf16, 512B/256B) at ~8ns/row of
   GpSimd issue time (vs 11ns/row for 128-row indirect DMAs).
 - Per block, one matmul with a host-built 0/1 segment matrix (lhsT, bf16)
   accumulates rows into the dst tile's PSUM bank: out[dst_row, feat] +=
   sum_p seg[p, dst_row] * g[p, feat]. 8 PSUM tiles live per supertile.
 - Self terms ride a direct DMA of the local shard + one f32 add at drain.
 - No shared-K ELL, no hop reordering: the hop pass (G3) runs in pi order,
   so scores fuse into its drain and v2's S11 realign pass is gone.

Stages (per core):
  S0   xW1' shard = (x_sh @ W1) * dis_sh (bf16) -> AG0 -> XW [NG,128]
  S2   T1 shard = [xW1'_loc | gather(XW, gperm)*ratio] ; AG1 -> T1 [NG,256]
  G1   blockmm pass over graph-g on T1 -> zd = relu(dis2 * s) -> AG2 -> ZD
  G2   blockmm pass over graph-g on ZD -> e1{,b} = relu(dis * (s_h @ W2));
       ship [e1*dishp|e1b*dishp] -> E1H ; e1*dis -> E1D ; MLP+tvec fused
  G4   blockmm pass over graph-g on E1D -> cls = (dis*s)@Wc -> out[:, :10]
  G3   blockmm pass over graph-h on E1H -> e2 = dishp * (s_h @ W3);
       scores = sigmoid(rowsum(tvec * e2{,b})) -> out[:, 10:12]
"""

import ml_dtypes
import numpy as np

import concourse.bacc as bacc
import concourse.bass as bass
import concourse.mybir as mybir
import concourse.tile as tile
from concourse import bass_utils
from concourse.masks import make_identity

P = 128
F32 = mybir.dt.float32
I16 = mybir.dt.int16
I32 = mybir.dt.int32
AF = mybir.ActivationFunctionType
ALU = mybir.AluOpType
TDT = mybir.dt.bfloat16  # transport/table dtype

N_WIN = 4
CH = 1024   # dma_gather cap (8 Q7 cores x 128 in-flight descriptors)
ST = 4      # dst tiles per supertile (PSUM banks are 2KB units)


# ----------------------------------------------------------------- host prep


def _wrap16(idx):
    """[n] -> [128, n//16] int16: value i at [i%16, i//16], replicated x8."""
    n = len(idx)
    w = np.ascontiguousarray(idx.reshape(n // 16, 16).T).astype(np.int16)
    return np.tile(w, (8, 1))


def _plane(vals_loc, nt):
    return np.ascontiguousarray(vals_loc.reshape(nt, P).T)


def _blk_build(per_core_edges, n_cores, nloc, ng):
    """Block structure + per-core gather-idx / segmat arrays.

    per_core_edges: [(src_gl, dst_loc)] WITHOUT self loops.
    Returns (sts, gidx_arrs, seg_arrs, total_rows, nblk):
      sts: list of {tiles: [(t, has_blocks)], chunks: [(w, nrows,
           [(tile, start, stop)])]}
      gidx_arrs: per-core [128, total//16] int16 (window-relative)
      seg_arrs: per-core [128, nblk*128] bf16 segment matrices
    """
    wrow = ng // N_WIN
    nt = nloc // P

    counts = np.zeros((n_cores, nt, N_WIN), np.int64)
    grouped = []
    for c, (src, dst) in enumerate(per_core_edges):
        w = src // wrow
        t = dst // P
        np.add.at(counts[c], (t, w), 1)
        o = np.lexsort((w, t))
        grouped.append((t[o], w[o], src[o], dst[o]))
    S = -(-counts.max(axis=0) // P) * P  # [nt, N_WIN] common padded sizes

    nst = -(-nt // ST)
    sts = []
    total = 0
    nblk = 0
    for si in range(nst):
        tiles = list(range(si * ST, min((si + 1) * ST, nt)))
        first_left = {t: True for t in tiles}
        # which block is the last for each tile: count blocks per tile
        blocks_of = {t: int(sum(S[t])) // P for t in tiles}
        seen = {t: 0 for t in tiles}
        chunks = []
        for w in range(N_WIN):
            run = [(t, int(S[t, w])) for t in tiles if S[t, w] > 0]
            # split into chunks of <= CH rows (128-aligned)
            pend = []
            pend_rows = 0
            for t, sz in run:
                while sz > 0:
                    take = min(sz, CH - pend_rows)
                    nb = take // P
                    for _ in range(nb):
                        st_f = first_left[t]
                        first_left[t] = False
                        seen[t] += 1
                        sp_f = seen[t] == blocks_of[t]
                        pend.append((t, st_f, sp_f))
                    pend_rows += take
                    sz -= take
                    if pend_rows == CH:
                        chunks.append((w, pend_rows, pend))
                        pend, pend_rows = [], 0
            if pend_rows:
                chunks.append((w, pend_rows, pend))
        for w, nrows, blks in chunks:
            total += nrows
            nblk += len(blks)
        sts.append(
            {
                "tiles": [(t, blocks_of[t] > 0) for t in tiles],
                "chunks": chunks,
            }
        )

    # per-core fills
    gidx_arrs, seg_arrs = [], []
    for c in range(n_cores):
        tg, wg, sg, dg = grouped[c]
        key = tg * N_WIN + wg
        # start offset of each (t, w) group in the sorted edge arrays
        starts = np.searchsorted(key, np.arange(nt * N_WIN) )
        ends = np.searchsorted(key, np.arange(nt * N_WIN) + 1)
        gflat = np.zeros(total, np.int64)
        segp = np.zeros(total, np.int64)   # partition index per row (pos%128)
        segc = np.full(total, -1, np.int64)  # dst col per row (-1 = pad)
        pos = 0
        for stv in sts:
            cursor = {}
            for w, nrows, blks in stv["chunks"]:
                for bj, (t, _sf, _pf) in enumerate(blks):
                    kk = t * N_WIN + w
                    cur = cursor.get(kk, starts[kk])
                    n = min(ends[kk] - cur, P)
                    base = pos + bj * P
                    if n > 0:
                        gflat[base : base + n] = sg[cur : cur + n] - w * wrow
                        segc[base : base + n] = dg[cur : cur + n] - t * P
                    cursor[kk] = cur + n
                    pos += 0
                pos += nrows
        assert pos == total
        blkid = np.arange(total) // P
        prow = np.arange(total) % P
        seg = np.zeros((P, nblk * P), ml_dtypes.bfloat16)
        m = segc >= 0
        seg[prow[m], blkid[m] * P + segc[m]] = 1.0
        gidx_arrs.append(_wrap16(gflat))
        seg_arrs.append(seg)

    return sts, gidx_arrs, seg_arrs, total, nblk


def prep(inputs, n_cores=8):
    x = np.asarray(inputs["x"], np.float32)
    ei = np.asarray(inputs["edge_index"], np.int64)
    eih = np.asarray(inputs["edge_index_hop"], np.int64)
    perm = np.asarray(inputs["perm"], np.int64)
    W1 = np.asarray(inputs["W1"], np.float32)
    W2 = np.asarray(inputs["W2"], np.float32)
    W3 = np.asarray(inputs["W3"], np.float32)
    M1 = np.asarray(inputs["M1"], np.float32)
    M2 = np.asarray(inputs["M2"], np.float32)
    Wc = np.asarray(inputs["Wc"], np.float32)
    Wd0 = np.asarray(inputs["Wd"], np.float32)[0]
    for bname in ("b1", "b2", "b3", "mb1", "mb2", "bc"):
        assert np.abs(np.asarray(inputs[bname])).max() == 0.0, (
            f"nonzero bias {bname} not supported by this kernel build"
        )

    N, n_feat = x.shape
    D = W1.shape[1]
    ncls = Wc.shape[1]
    max_real = -(-N // n_cores)
    nloc = -(-(max_real + 1) // P) * P
    nt = nloc // P
    ng = n_cores * nloc
    assert ng % N_WIN == 0 and ng // N_WIN < 2**15

    deg = np.bincount(ei[1], minlength=N).astype(np.float32) + 1.0
    degh = np.bincount(eih[1], minlength=N).astype(np.float32) + 1.0
    dis = 1.0 / np.sqrt(deg)
    dish = 1.0 / np.sqrt(degh)

    order = np.argsort(-deg, kind="stable")
    core_of = np.empty(N, np.int64)
    loc_of = np.empty(N, np.int64)
    core_of[order] = np.arange(N) % n_cores
    loc_of[order] = np.arange(N) // n_cores
    gl = core_of * nloc + loc_of
    padrow = [c * nloc + nloc - 1 for c in range(n_cores)]

    nat = np.full((n_cores, nloc), -1, np.int64)
    nat[core_of, loc_of] = np.arange(N)

    def edge_lists(e2n):
        src, dst = e2n
        return [
            (gl[src[core_of[dst] == c]], loc_of[dst[core_of[dst] == c]])
            for c in range(n_cores)
        ]

    sts_g, gidx_g, seg_g, tot_g, nblk_g = _blk_build(
        edge_lists(ei), n_cores, nloc, ng
    )
    sts_h, gidx_h, seg_h, tot_h, nblk_h = _blk_build(
        edge_lists(eih), n_cores, nloc, ng
    )

    in_maps = []
    for c in range(n_cores):
        natc = nat[c]
        real = natc >= 0
        xs = np.zeros((nloc, n_feat), np.float32)
        xs[real] = x[natc[real]]
        dis_c = np.ones(nloc, np.float32)
        dis_c[real] = dis[natc[real]]
        dishp = np.ones(nloc, np.float32)
        dishp[real] = dish[natc[real]]
        gperm = np.full(nloc, padrow[c], np.int64)
        ratio = np.ones(nloc, np.float32)
        pv = perm[natc[real]]
        gperm[real] = gl[pv]
        ratio[real] = dis[natc[real]] / dis[pv]
        in_maps.append(
            {
                "xT": np.ascontiguousarray(xs.T).astype(ml_dtypes.bfloat16),
                "dis_p": _plane(dis_c, nt),
                "dis2_p": _plane(dis_c * dis_c, nt),
                "dishp_p": _plane(dishp, nt),
                "ratio_p": _plane(ratio, nt),
                "gperm_p": _plane(gperm.astype(np.int32), nt),
                "gidx_g": gidx_g[c],
                "seg_g": seg_g[c],
                "gidx_h": gidx_h[c],
                "seg_h": seg_h[c],
                "W1": W1.astype(ml_dtypes.bfloat16),
                "W2": W2,
                "W3": W3,
                "M1": M1,
                "M2": M2,
                "Wd0": Wd0,
                "Wc": np.ascontiguousarray(Wc),
            }
        )

    meta = dict(
        n_cores=n_cores,
        nloc=nloc,
        nt=nt,
        ng=ng,
        n_feat=n_feat,
        D=D,
        ncls=ncls,
        sts_g=sts_g,
        sts_h=sts_h,
        tot_g=tot_g,
        tot_h=tot_h,
        nblk_g=nblk_g,
        nblk_h=nblk_h,
        core_of=core_of,
        loc_of=loc_of,
    )
    return in_maps, meta


# ------------------------------------------------------------- device build


def build(meta):
    n_cores = meta["n_cores"]
    nloc, nt, ng = meta["nloc"], meta["nt"], meta["ng"]
    n_feat, D, ncls = meta["n_feat"], meta["D"], meta["ncls"]
    DD = 2 * D
    nfc = n_feat // P
    wrow = ng // N_WIN
    groups = [list(range(n_cores))]

    nc = bacc.Bacc("TRN2", debug=False, num_devices=n_cores)
    shared = "Shared" if n_cores > 4 else "Local"

    xT = nc.dram_tensor("xT", [n_feat, nloc], TDT, kind="ExternalInput")
    dis_p = nc.dram_tensor("dis_p", [P, nt], F32, kind="ExternalInput")
    dis2_p = nc.dram_tensor("dis2_p", [P, nt], F32, kind="ExternalInput")
    dishp_p = nc.dram_tensor("dishp_p", [P, nt], F32, kind="ExternalInput")
    ratio_p = nc.dram_tensor("ratio_p", [P, nt], F32, kind="ExternalInput")
    gperm_p = nc.dram_tensor("gperm_p", [P, nt], I32, kind="ExternalInput")
    gidx_g = nc.dram_tensor("gidx_g", [P, meta["tot_g"] // 16], I16, kind="ExternalInput")
    seg_g = nc.dram_tensor("seg_g", [P, meta["nblk_g"] * P], TDT, kind="ExternalInput")
    gidx_h = nc.dram_tensor("gidx_h", [P, meta["tot_h"] // 16], I16, kind="ExternalInput")
    seg_h = nc.dram_tensor("seg_h", [P, meta["nblk_h"] * P], TDT, kind="ExternalInput")
    W1 = nc.dram_tensor("W1", [n_feat, D], TDT, kind="ExternalInput")
    W2 = nc.dram_tensor("W2", [D, D], F32, kind="ExternalInput")
    W3 = nc.dram_tensor("W3", [D, D], F32, kind="ExternalInput")
    M1 = nc.dram_tensor("M1", [D, D], F32, kind="ExternalInput")
    M2 = nc.dram_tensor("M2", [D, D], F32, kind="ExternalInput")
    Wd0 = nc.dram_tensor("Wd0", [D, D], F32, kind="ExternalInput")
    Wc = nc.dram_tensor("Wc", [D, ncls], F32, kind="ExternalInput")
    out = nc.dram_tensor("out", [nloc, ncls + 2], F32, kind="ExternalOutput")

    xw_s = nc.dram_tensor("xw_s", [nloc, D], TDT, kind="Internal")
    XW = nc.dram_tensor("XW", [ng, D], TDT, kind="Internal", addr_space=shared)
    t1_s = nc.dram_tensor("t1_s", [nloc, DD], TDT, kind="Internal")
    T1 = nc.dram_tensor("T1", [ng, DD], TDT, kind="Internal", addr_space=shared)
    zd_s = nc.dram_tensor("zd_s", [nloc, DD], TDT, kind="Internal")
    ZD = nc.dram_tensor("ZD", [ng, DD], TDT, kind="Internal", addr_space=shared)
    e1h_s = nc.dram_tensor("e1h_s", [nloc, DD], TDT, kind="Internal")
    e1d_s = nc.dram_tensor("e1d_s", [nloc, D], TDT, kind="Internal")
    E1H = nc.dram_tensor("E1H", [ng, DD], TDT, kind="Internal", addr_space=shared)
    E1D = nc.dram_tensor("E1D", [ng, D], TDT, kind="Internal", addr_space=shared)
    TV = nc.dram_tensor("TV", [nloc, D], F32, kind="Internal")

    with tile.TileContext(nc) as tc:
        with (
            tc.tile_pool(name="const", bufs=1) as constp,
            tc.tile_pool(name="gath", bufs=4) as gathp,
            tc.tile_pool(name="segp", bufs=4) as segp,
            tc.tile_pool(name="idx", bufs=4) as idxp,
            tc.tile_pool(name="work", bufs=3) as workp,
            tc.tile_pool(name="outp", bufs=3) as outp,
            tc.tile_pool(name="psum", bufs=2, space="PSUM") as psp,
        ):
            ident = constp.tile([P, P], F32)
            make_identity(nc, ident[:])

            def res(t_dram, w, dt=F32, name=None):
                tl = constp.tile([P, w], dt, name=name)
                nc.sync.dma_start(tl[:], t_dram.ap())
                return tl

            disq = res(dis_p, nt, name="disq")
            dis2q = res(dis2_p, nt, name="dis2q")
            dishpq = res(dishp_p, nt, name="dishpq")
            ratioq = res(ratio_p, nt, name="ratioq")
            gpermq = res(gperm_p, nt, I32, name="gpermq")

            w1t = [constp.tile([P, D], TDT, name=f"w1t_{i}") for i in range(nfc)]
            for i in range(nfc):
                nc.sync.dma_start(w1t[i][:], W1.ap()[i * P : (i + 1) * P])
            w2t = res(W2, D, name="w2t")
            w3t = res(W3, D, name="w3t")
            m1t = res(M1, D, name="m1t")
            m2t = res(M2, D, name="m2t")
            wdt = res(Wd0, D, name="wdt")
            wct = res(Wc, ncls, name="wct")

            def rows(t):
                return slice(t * P, (t + 1) * P)

            def col(plane, t):
                return plane[:, t : t + 1]

            # ---- blockmm pass: gather chunks + segmat matmuls into PSUM
            def run_pass(table, W, sts, gidx_t, seg_t, self_dram, compute_fn):
                colpos = 0
                blkpos = 0
                for stv in sts:
                    pts = {}
                    for t, has in stv["tiles"]:
                        if has:
                            pt = psp.tile(
                                [P, DD], F32, tag=f"pt{t % ST}", bufs=1,
                                name=f"pt{t % ST}",
                            )
                            pts[t] = pt
                    for w, nrows, blks in stv["chunks"]:
                        gi = idxp.tile([P, CH // 16], I16, tag="gi")
                        nc.sync.dma_start(
                            gi[:, : nrows // 16],
                            gidx_t.ap()[:, colpos : colpos + nrows // 16],
                        )
                        slab = segp.tile([P, (CH // P) * P], TDT, tag="slab")
                        nc.sync.dma_start(
                            slab[:, : len(blks) * P],
                            seg_t.ap()[:, blkpos * P : (blkpos + len(blks)) * P],
                        )
                        g = gathp.tile([P, (CH // P) * DD], TDT, tag="g")
                        gv = g[:, : (nrows // P) * W].rearrange(
                            "p (b w) -> p b w", w=W
                        )
                        nc.gpsimd.dma_gather(
                            out_ap=gv,
                            in_ap=table.ap()[w * wrow : (w + 1) * wrow],
                            idxs_ap=gi[:, : nrows // 16],
                            num_idxs=nrows,
                            num_idxs_reg=nrows,
                            elem_size=W,
                        )
                        for j, (t, st_f, sp_f) in enumerate(blks):
                            nc.tensor.matmul(
                                out=pts[t][:, :W],
                                lhsT=slab[:, j * P : (j + 1) * P],
                                rhs=gv[:, j, :],
                                start=st_f,
                                stop=sp_f,
                            )
                        colpos += nrows // 16
                        blkpos += len(blks)
                    for t, has in stv["tiles"]:
                        lt = workp.tile([P, W], TDT, tag="selfl")
                        nc.sync.dma_start(lt[:], self_dram.ap()[rows(t)])
                        s = workp.tile([P, W], F32, tag="se")
                        if has:
                            nc.vector.tensor_tensor(
                                out=s[:], in0=pts[t][:, :W], in1=lt[:], op=ALU.add
                            )
                        else:
                            nc.vector.tensor_copy(s[:], lt[:])
                        compute_fn(t, s)

            # ---- S0: xW1' shard (two tiles per iteration, shared loads)
            for t0 in range(0, nt, 2):
                pss = [psp.tile([P, D], F32, tag="m", bufs=2, name=f"s0ps{h}") for h in range(2)]
                for i in range(nfc):
                    xt = workp.tile([P, 2 * P], TDT, tag="xt")
                    nc.sync.dma_start(
                        xt[:], xT.ap()[i * P : (i + 1) * P, t0 * P : (t0 + 2) * P]
                    )
                    for h in range(2):
                        nc.tensor.matmul(
                            out=pss[h][:],
                            lhsT=xt[:, h * P : (h + 1) * P],
                            rhs=w1t[i][:],
                            start=(i == 0),
                            stop=(i == nfc - 1),
                        )
                for h in range(2):
                    o = outp.tile([P, D], TDT, tag="s0")
                    nc.vector.tensor_scalar_mul(o[:], pss[h][:], col(disq, t0 + h))
                    nc.sync.dma_start(xw_s.ap()[rows(t0 + h)], o[:])

            nc.gpsimd.collective_compute(
                "AllGather",
                ALU.bypass,
                replica_groups=groups,
                ins=[xw_s[:].opt()],
                outs=[XW[:].opt()],
            )

            # ---- S2: T1 shard (perm gather via indirect DMA, deg=1)
            for t in range(nt):
                gp_ = gathp.tile([P, D], TDT, tag="g2")
                nc.gpsimd.indirect_dma_start(
                    out=gp_[:],
                    out_offset=None,
                    in_=XW.ap(),
                    in_offset=bass.IndirectOffsetOnAxis(ap=col(gpermq, t), axis=0),
                )
                o = outp.tile([P, D], TDT, tag="s2")
                nc.vector.tensor_scalar_mul(o[:], gp_[:], col(ratioq, t))
                nc.sync.dma_start(t1_s.ap()[rows(t), D:DD], o[:])
                l = workp.tile([P, D], TDT, tag="s2l")
                nc.sync.dma_start(l[:], xw_s.ap()[rows(t)])
                nc.sync.dma_start(t1_s.ap()[rows(t), 0:D], l[:])

            nc.gpsimd.collective_compute(
                "AllGather",
                ALU.bypass,
                replica_groups=groups,
                ins=[t1_s[:].opt()],
                outs=[T1[:].opt()],
            )

            # ---- G1
            def g1_compute(t, s):
                o = outp.tile([P, DD], TDT, tag="ze")
                nc.vector.tensor_scalar(
                    o[:], s[:], col(dis2q, t), 0.0, ALU.mult, ALU.max
                )
                nc.sync.dma_start(zd_s.ap()[rows(t)], o[:])

            run_pass(T1, DD, meta["sts_g"], gidx_g, seg_g, t1_s, g1_compute)

            nc.gpsimd.collective_compute(
                "AllGather",
                ALU.bypass,
                replica_groups=groups,
                ins=[zd_s[:].opt()],
                outs=[ZD[:].opt()],
            )

            # ---- G2 (+ fused MLP/tvec)
            def g2_compute(t, s):
                e1h = outp.tile([P, DD], TDT, tag="e1h")
                e1d = outp.tile([P, D], TDT, tag="e1d")
                e1p = outp.tile([P, D], F32, tag="e1p")
                for h in range(2):
                    tp = psp.tile([P, P], F32, tag="t", bufs=2)
                    nc.tensor.transpose(
                        out=tp[:], in_=s[:, h * D : (h + 1) * D], identity=ident[:]
                    )
                    tps = workp.tile([P, P], F32, tag="tps")
                    nc.vector.tensor_copy(tps[:], tp[:])
                    mm = psp.tile([P, D], F32, tag="m", bufs=2)
                    nc.tensor.matmul(
                        out=mm[:], lhsT=tps[:], rhs=w2t[:], start=True, stop=True
                    )
                    eh = workp.tile([P, D], F32, tag="eh")
                    nc.vector.tensor_scalar(
                        eh[:], mm[:], col(disq, t), 0.0, ALU.mult, ALU.max
                    )
                    nc.vector.tensor_scalar_mul(
                        e1h[:, h * D : (h + 1) * D], eh[:], col(dishpq, t)
                    )
                    if h == 0:
                        nc.vector.tensor_copy(e1p[:], eh[:])
                        nc.vector.tensor_scalar_mul(e1d[:], eh[:], col(disq, t))
                nc.sync.dma_start(e1h_s.ap()[rows(t)], e1h[:])
                nc.sync.dma_start(e1d_s.ap()[rows(t)], e1d[:])
                tpa = psp.tile([P, P], F32, tag="t", bufs=2)
                nc.tensor.transpose(out=tpa[:], in_=e1p[:], identity=ident[:])
                tpsa = workp.tile([P, P], F32, tag="tps")
                nc.vector.tensor_copy(tpsa[:], tpa[:])
                mma = psp.tile([P, D], F32, tag="m", bufs=2)
                nc.tensor.matmul(out=mma[:], lhsT=tpsa[:], rhs=m1t[:], start=True, stop=True)
                u = workp.tile([P, D], F32, tag="ml_u")
                nc.scalar.activation(u[:], mma[:], AF.Relu)
                tpb = psp.tile([P, P], F32, tag="t", bufs=2)
                nc.tensor.transpose(out=tpb[:], in_=u[:], identity=ident[:])
                tpsb = workp.tile([P, P], F32, tag="tps")
                nc.vector.tensor_copy(tpsb[:], tpb[:])
                mmb = psp.tile([P, D], F32, tag="m", bufs=2)
                nc.tensor.matmul(out=mmb[:], lhsT=tpsb[:], rhs=m2t[:], start=True, stop=True)
                e3 = workp.tile([P, D], F32, tag="ml_e3")
                nc.vector.tensor_copy(e3[:], mmb[:])
                tpc = psp.tile([P, P], F32, tag="t", bufs=2)
                nc.tensor.transpose(out=tpc[:], in_=e3[:], identity=ident[:])
                tpsc = workp.tile([P, P], F32, tag="tps")
                nc.vector.tensor_copy(tpsc[:], tpc[:])
                mmc = psp.tile([P, D], F32, tag="m", bufs=2)
                nc.tensor.matmul(out=mmc[:], lhsT=tpsc[:], rhs=wdt[:], start=True, stop=True)
                tv = outp.tile([P, D], F32, tag="ml_tv")
                nc.vector.tensor_copy(tv[:], mmc[:])
                nc.sync.dma_start(TV.ap()[rows(t)], tv[:])

            run_pass(ZD, DD, meta["sts_g"], gidx_g, seg_g, zd_s, g2_compute)

            nc.gpsimd.collective_compute(
                "AllGather",
                ALU.bypass,
                replica_groups=groups,
                ins=[e1d_s[:].opt()],
                outs=[E1D[:].opt()],
            )
            nc.gpsimd.collective_compute(
                "AllGather",
                ALU.bypass,
                replica_groups=groups,
                ins=[e1h_s[:].opt()],
                outs=[E1H[:].opt()],
            )

            # ---- G4 (before G3: its gathers hide the E1H AllGather)
            def g4_compute(t, s):
                sc_ = workp.tile([P, D], F32, tag="c_s")
                nc.vector.tensor_scalar_mul(sc_[:], s[:], col(disq, t))
                tp = psp.tile([P, P], F32, tag="t", bufs=2)
                nc.tensor.transpose(out=tp[:], in_=sc_[:], identity=ident[:])
                tps = workp.tile([P, P], F32, tag="tps")
                nc.vector.tensor_copy(tps[:], tp[:])
                mm = psp.tile([P, ncls], F32, tag="m", bufs=2)
                nc.tensor.matmul(out=mm[:], lhsT=tps[:], rhs=wct[:], start=True, stop=True)
                o = outp.tile([P, ncls], F32, tag="c_o")
                nc.vector.tensor_copy(o[:], mm[:])
                nc.sync.dma_start(out.ap()[rows(t), 0:ncls], o[:])

            run_pass(E1D, D, meta["sts_g"], gidx_g, seg_g, e1d_s, g4_compute)

            # ---- G3 (+ fused scores)
            def g3_compute(t, s):
                e2 = workp.tile([P, DD], F32, tag="e2")
                for h in range(2):
                    tp = psp.tile([P, P], F32, tag="t", bufs=2)
                    nc.tensor.transpose(
                        out=tp[:], in_=s[:, h * D : (h + 1) * D], identity=ident[:]
                    )
                    tps = workp.tile([P, P], F32, tag="tps")
                    nc.vector.tensor_copy(tps[:], tp[:])
                    mm = psp.tile([P, D], F32, tag="m", bufs=2)
                    nc.tensor.matmul(
                        out=mm[:], lhsT=tps[:], rhs=w3t[:], start=True, stop=True
                    )
                    nc.vector.tensor_scalar_mul(
                        e2[:, h * D : (h + 1) * D], mm[:], col(dishpq, t)
                    )
                tv = workp.tile([P, D], F32, tag="sc_tv")
                nc.sync.dma_start(tv[:], TV.ap()[rows(t)])
                pr = workp.tile([P, DD], F32, tag="sc_pr")
                nc.vector.tensor_mul(pr[:, 0:D], tv[:], e2[:, 0:D])
                nc.vector.tensor_mul(pr[:, D:DD], tv[:], e2[:, D:DD])
                rs = workp.tile([P, 2], F32, tag="sc_rs")
                nc.vector.tensor_reduce(
                    out=rs[:],
                    in_=pr[:].rearrange("p (h d) -> p h d", h=2),
                    axis=mybir.AxisListType.X,
                    op=ALU.add,
                )
                sg = outp.tile([P, 2], F32, tag="sc_sg")
                nc.scalar.activation(sg[:], rs[:], AF.Sigmoid)
                nc.sync.dma_start(out.ap()[rows(t), ncls : ncls + 2], sg[:])

            run_pass(E1H, DD, meta["sts_h"], gidx_h, seg_h, e1h_s, g3_compute)

    nc.compile()
    return nc


def assemble(results, meta):
    n_cores = meta["n_cores"]
    N = len(meta["core_of"])
    ncls = meta["ncls"]
    out = np.empty((N, ncls + 2), np.float32)
    for c in range(n_cores):
        oc = results[c]["out"]
        m = meta["core_of"] == c
        out[m] = oc[meta["loc_of"][m]]
    return out


# ------------------------------------------------------------------ entry


_CACHE = {}


def kernel(**inputs):
    """Full-input entry point: shards across 8 NeuronCores internally.

    Expects the nn_MixModel input dict (x, edge_index, edge_index_hop, y,
    perm, W1..Wd); returns the full [N, n_cls+2] float32 output.
    """
    n_cores = 8
    in_maps, meta = prep(inputs, n_cores)
    key = (meta["nloc"], str(meta["sts_g"]), str(meta["sts_h"]))
    nc = _CACHE.get(key)
    if nc is None:
        nc = build(meta)
        _CACHE[key] = nc
    res = bass_utils.run_bass_kernel_spmd(
        nc, in_maps, core_ids=list(range(n_cores)), trace=False
    )
    return assemble(res.results, meta)
